# revision 9
# baseline (speedup 1.0000x reference)
"""MoE FFN (top-2 routing) Trainium2 kernel.

Strategy (8 NeuronCores, SPMD via run_bass_kernel_spmd):
  Pass 1 (router, data-parallel over tokens): each core takes N/8 = 512
    tokens (x pre-transposed to [D, 512] by the host), computes logits =
    x @ gate_w in fp32 on the PE (gate stationary, tokens moving, then a
    PE transpose back to token-partition layout), derives the top-2
    expert mask and softmax weights on-device with batched DVE ops, and
    emits:
      - wfull [512, E]: per-token router weight for every expert
        (nonzero exactly at the token's top-2 experts)
      - aux partials [1, 17]: softmax-prob column sums (8), top-1
        one-hot column sums (8), sum of logits^2 (1)
  Host dispatch ("all-to-all"): tokens are gathered per expert from the
    nonzero pattern of wfull, laid out transposed ([D, C], zero-padded
    to capacity C), and shipped to the expert's core. Pure data
    movement - no arithmetic on the host.
  Pass 2 (expert FFN, expert-parallel): core e holds expert e's w1/w2
    resident in SBUF and streams its gathered tokens through
      hT = gelu(w1.T @ xT + b1)   [F on partitions]
      y  = wtok * (hT.T @ w2 + b2) [tokens on partitions]
    with float32r matmuls (fp32 data rounded to 11-bit mantissa, 1
    cycle/row on the PE for free dim >= 256). Weights are split into
    per-chunk tiles and DMA-ordered so the PE starts as soon as the
    first chunks land. Also combines the pass-1 aux partials into the
    scalar aux loss on-device.
  Host combine: scatter-add of the (already router-weighted) per-expert
    outputs back to [B, T, D]. Each token receives exactly its two
    expert contributions.
"""

import os
import numpy as np

import concourse.bass as bass
import concourse.mybir as mybir
import concourse.tile as tile
from concourse import bacc
from concourse.alu_op_type import AluOpType
from concourse.bass_utils import run_bass_kernel_spmd
from concourse.masks import make_identity

f32 = mybir.dt.float32
f32r = mybir.dt.float32r
AX = mybir.AxisListType
ACT = mybir.ActivationFunctionType
TT = AluOpType

B, T, D, E, F = 2, 2048, 1024, 8, 2048
N = B * T           # 4096 tokens
NCORES = 8
TPC = N // NCORES   # 512 tokens per core in pass 1
DCH = D // 128      # 8 d-chunks
FCH = F // 128      # 16 f-chunks
TBLK = 384          # pass-2 token block (moving dim; >=256 keeps f32r fast)

Z_LOSS_COEF = 1e-3

_nc_cache = {}


def _build_pass1():
    """Router kernel: one core's 512-token shard."""
    nc = bacc.Bacc("TRN2", target_bir_lowering=False, debug=False,
                   num_devices=NCORES)
    xt_d = nc.dram_tensor("xt", [D, TPC], f32, kind="ExternalInput").ap()
    gwt_d = nc.dram_tensor("gwt", [128, DCH * E], f32, kind="ExternalInput").ap()
    wfull_d = nc.dram_tensor("wfull", [TPC, E], f32, kind="ExternalOutput").ap()
    auxp_d = nc.dram_tensor("auxp", [1, 17], f32, kind="ExternalOutput").ap()

    NB = TPC // 128  # 4 token blocks per core

    with tile.TileContext(nc) as tc:
        with tc.tile_pool(name="sb", bufs=1) as sb, \
             tc.tile_pool(name="ps", bufs=2, space="PSUM") as ps, \
             tc.tile_pool(name="pl", bufs=1, space="PSUM") as pl:
            gw = sb.tile([128, DCH, E], f32)
            nc.gpsimd.dma_start(out=gw[:], in_=gwt_d.rearrange("p (c e) -> p c e", e=E))
            xts = []
            for d in range(DCH):
                xtd = sb.tile([128, TPC], f32, name=f"xt{d}", tag=f"xt{d}")
                nc.sync.dma_start(out=xtd[:], in_=xt_d[bass.ts(d, 128), :])
                xts.append(xtd)
            ident = sb.tile([128, 128], f32)
            make_identity(nc, ident[:])
            ones = sb.tile([128, 1], f32)
            nc.vector.memset(ones[:], 1.0)

            # logits^T [E, TPC] on PSUM (gate stationary, tokens moving, fp32)
            lgT = pl.tile([E, TPC], f32, tag="lgT")
            for d in range(DCH):
                nc.tensor.matmul(lgT[:], lhsT=gw[:, d, :], rhs=xts[d][:],
                                 start=(d == 0), stop=(d == DCH - 1))
            lgTs = sb.tile([E, TPC], f32)
            nc.scalar.copy(lgTs[:], lgT[:])
            # transpose back to [128 tokens, E] per block, gather into SBUF
            lgs = sb.tile([128, NB, E], f32)
            for b in range(NB):
                lgp = ps.tile([128, E], f32, tag="lgp")
                nc.tensor.transpose(lgp[:], lgTs[:, bass.ts(b, 128)],
                                    ident[:E, :E])
                nc.scalar.copy(lgs[:, b, :], lgp[:])

            def bc(ap):  # [128, NB] -> [128, NB, E] stride-0 broadcast
                return ap[:, :, None].broadcast_to([128, NB, E])

            acc = sb.tile([128, 17], f32)   # [probs 8 | onehot 8 | z 1]
            t1 = sb.tile([128, NB], f32)
            nc.vector.tensor_reduce(t1[:], lgs[:], axis=AX.X, op=TT.max)
            eq1 = sb.tile([128, NB, E], f32)
            nc.vector.tensor_tensor(eq1[:], lgs[:], bc(t1), TT.is_equal)
            msk = sb.tile([128, NB, E], f32)
            nc.vector.scalar_tensor_tensor(out=msk[:], in0=eq1[:], scalar=-1e30,
                                           in1=lgs[:], op0=TT.mult, op1=TT.add)
            t2 = sb.tile([128, NB], f32)
            nc.vector.tensor_reduce(t2[:], msk[:], axis=AX.X, op=TT.max)
            eq2 = sb.tile([128, NB, E], f32)
            nc.vector.tensor_tensor(eq2[:], msk[:], bc(t2), TT.is_equal)
            # top-2 softmax weights: wa = 1/(1+exp(t2-t1)), wb = 1-wa
            d21 = sb.tile([128, NB], f32)
            nc.vector.tensor_sub(d21[:], t2[:], t1[:])
            ex = sb.tile([128, NB], f32)
            nc.scalar.activation(ex[:], d21[:], ACT.Exp)
            den = sb.tile([128, NB], f32)
            nc.vector.tensor_scalar_add(den[:], ex[:], 1.0)
            wa = sb.tile([128, NB], f32)
            nc.vector.reciprocal(wa[:], den[:])
            wb = sb.tile([128, NB], f32)
            nc.vector.tensor_mul(wb[:], ex[:], wa[:])
            # wfull = wa*eq1 + wb*eq2
            wf = sb.tile([128, NB, E], f32)
            nc.vector.tensor_tensor(wf[:], eq1[:], bc(wa), TT.mult)
            wf2 = sb.tile([128, NB, E], f32)
            nc.vector.tensor_tensor(wf2[:], eq2[:], bc(wb), TT.mult)
            nc.vector.tensor_add(wf[:], wf[:], wf2[:])
            nc.sync.dma_start(out=wfull_d.rearrange("(b p) e -> p b e", p=128),
                              in_=wf[:])
            # softmax probs for aux
            sub = sb.tile([128, NB, E], f32)
            nc.vector.tensor_tensor(sub[:], lgs[:], bc(t1), TT.subtract)
            pr = sb.tile([128, NB, E], f32)
            nc.scalar.activation(pr[:], sub[:], ACT.Exp)
            se = sb.tile([128, NB], f32)
            nc.vector.tensor_reduce(se[:], pr[:], axis=AX.X, op=TT.add)
            rs = sb.tile([128, NB], f32)
            nc.vector.reciprocal(rs[:], se[:])
            prn = sb.tile([128, NB, E], f32)
            nc.vector.tensor_tensor(prn[:], pr[:], bc(rs), TT.mult)
            # block-sums into acc
            nc.vector.tensor_add(acc[:, 0:E], prn[:, 0, :], prn[:, 1, :])
            nc.vector.tensor_add(acc[:, 0:E], acc[:, 0:E], prn[:, 2, :])
            nc.vector.tensor_add(acc[:, 0:E], acc[:, 0:E], prn[:, 3, :])
            nc.vector.tensor_add(acc[:, E:2 * E], eq1[:, 0, :], eq1[:, 1, :])
            nc.vector.tensor_add(acc[:, E:2 * E], acc[:, E:2 * E], eq1[:, 2, :])
            nc.vector.tensor_add(acc[:, E:2 * E], acc[:, E:2 * E], eq1[:, 3, :])
            sq = sb.tile([128, NB, E], f32)
            nc.scalar.activation(sq[:], lgs[:], ACT.Square,
                                 accum_out=acc[:, 16:17])
            # column sums over the 128 partitions via ones-matmul
            accp = ps.tile([1, 17], f32, tag="accp")
            nc.tensor.matmul(accp[:], lhsT=ones[:], rhs=acc[:], start=True,
                             stop=True)
            accs = sb.tile([1, 17], f32)
            nc.vector.tensor_copy(accs[:], accp[:])
            nc.sync.dma_start(out=auxp_d[:], in_=accs[:])

    nc.compile()
    return nc


def _build_pass2(C):
    """Expert FFN kernel: one expert's C gathered tokens (C % TBLK == 0)."""
    nc = bacc.Bacc("TRN2", target_bir_lowering=False, debug=False,
                   num_devices=NCORES)
    xgt_d = nc.dram_tensor("xgt", [D, C], f32r, kind="ExternalInput").ap()
    w1_d = nc.dram_tensor("w1", [D, F], f32r, kind="ExternalInput").ap()
    w2_d = nc.dram_tensor("w2", [F, D], f32r, kind="ExternalInput").ap()
    b1t_d = nc.dram_tensor("b1t", [128, FCH], f32, kind="ExternalInput").ap()
    b2b_d = nc.dram_tensor("b2b", [128, D], f32, kind="ExternalInput").ap()
    wtok_d = nc.dram_tensor("wtok", [128, C // 128], f32, kind="ExternalInput").ap()
    auxp_d = nc.dram_tensor("auxp", [NCORES, 17], f32, kind="ExternalInput").ap()
    y_d = nc.dram_tensor("y", [C, D], f32, kind="ExternalOutput").ap()
    aux_d = nc.dram_tensor("aux", [1, 1], f32, kind="ExternalOutput").ap()

    nblk = C // TBLK
    nsub = TBLK // 128

    with tile.TileContext(nc) as tc:
        with tc.tile_pool(name="wt", bufs=1) as wt, \
             tc.tile_pool(name="xg", bufs=2) as xg, \
             tc.tile_pool(name="hp", bufs=1) as hp, \
             tc.tile_pool(name="yo", bufs=2) as yo, \
             tc.tile_pool(name="sm", bufs=1) as sm, \
             tc.tile_pool(name="ph", bufs=6, space="PSUM") as ph, \
             tc.tile_pool(name="py", bufs=2, space="PSUM") as py:

            def xgt_block(t):
                xts = []
                for d in range(DCH):
                    xtd = xg.tile([128, TBLK], f32r, name=f"xt{t}_{d}",
                                  tag=f"xtd{d}")
                    nc.sync.dma_start(
                        out=xtd[:],
                        in_=xgt_d[bass.ts(d, 128), bass.ts(t, TBLK)])
                    xts.append(xtd)
                return xts

            # DMA issue order: first block's tokens + w1 first (mm1 needs
            # them), then small constants, then w2 (mm2 starts ~25us in).
            xts0 = xgt_block(0)
            w1c = []
            for d in range(DCH):
                w1d = wt.tile([128, F], f32r, name=f"w1_{d}")
                nc.sync.dma_start(out=w1d[:], in_=w1_d[bass.ts(d, 128), :])
                w1c.append(w1d)
            b1t = wt.tile([128, FCH], f32)
            nc.sync.dma_start(out=b1t[:], in_=b1t_d[:])
            wtok = wt.tile([128, C // 128], f32)
            nc.sync.dma_start(out=wtok[:], in_=wtok_d[:])
            w2c = []
            for f in range(FCH):
                w2f = wt.tile([128, D], f32r, name=f"w2_{f}")
                nc.sync.dma_start(out=w2f[:], in_=w2_d[bass.ts(f, 128), :])
                w2c.append(w2f)
            b2b = wt.tile([128, D], f32)
            nc.sync.dma_start(out=b2b[:], in_=b2b_d[:])

            # aux combine (tiny, once)
            auxp = sm.tile([NCORES, 17], f32)
            nc.sync.dma_start(out=auxp[:], in_=auxp_d[:])
            ones8 = sm.tile([NCORES, 1], f32)
            nc.vector.memset(ones8[:], 1.0)
            auxs = sm.tile([1, 17], f32)
            auxt = py.tile([1, 17], f32, tag="yps", name="auxt")
            nc.tensor.matmul(auxt[:], lhsT=ones8[:], rhs=auxp[:], start=True,
                             stop=True)
            nc.vector.tensor_copy(auxs[:], auxt[:])
            prod = sm.tile([1, E], f32)
            nc.vector.tensor_mul(prod[:], auxs[:, 0:E], auxs[:, E:2 * E])
            psum_ = sm.tile([1, 1], f32)
            nc.vector.tensor_reduce(psum_[:], prod[:], axis=AX.X, op=TT.add)
            zt = sm.tile([1, 1], f32)
            nc.vector.tensor_scalar_mul(zt[:], auxs[:, 16:17],
                                        float(Z_LOSS_COEF / (N * E)))
            auxo = sm.tile([1, 1], f32)
            nc.vector.scalar_tensor_tensor(out=auxo[:], in0=psum_[:],
                                           scalar=float(E) / (float(N) * float(N)),
                                           in1=zt[:], op0=TT.mult, op1=TT.add)
            nc.sync.dma_start(out=aux_d[:], in_=auxo[:])

            # main FFN loop. mm1 runs d-outer over f-groups of <=6 so the
            # PE consumes w1/xgt chunks in DMA-arrival order on block 0.
            FG = [list(range(0, 6)), list(range(6, 12)), list(range(12, 16))]
            for t in range(nblk):
                xts = xts0 if t == 0 else xgt_block(t)
                hT = hp.tile([128, FCH, TBLK], f32r, tag="hT")
                for fg in FG:
                    hps_l = [ph.tile([128, TBLK], f32, tag="hps",
                                     name=f"hps{t}_{f}") for f in fg]
                    for d in range(DCH):
                        for j, f in enumerate(fg):
                            nc.tensor.matmul(hps_l[j][:],
                                             lhsT=w1c[d][:, bass.ts(f, 128)],
                                             rhs=xts[d][:],
                                             start=(d == 0), stop=(d == DCH - 1))
                    # hT = gelu(w1.T x + b1), erf flavor
                    for j, f in enumerate(fg):
                        nc.scalar.activation(hT[:, f, :], hps_l[j][:], ACT.Gelu,
                                             bias=b1t[:, f:f + 1])
                for s in range(nsub):
                    yt = yo.tile([128, D], f32, tag="yt")
                    for n in range(2):
                        yps = py.tile([128, 512], f32, tag="yps")
                        for f in range(FCH):
                            nc.tensor.matmul(yps[:],
                                             lhsT=hT[:, f, bass.ts(s, 128)],
                                             rhs=w2c[f][:, bass.ts(n, 512)],
                                             start=(f == 0), stop=(f == FCH - 1))
                        # y = wtok * (psum + b2)
                        tb = yo.tile([128, 512], f32, tag="tb")
                        nc.vector.tensor_add(tb[:], yps[:], b2b[:, bass.ts(n, 512)])
                        nc.scalar.activation(
                            yt[:, bass.ts(n, 512)], tb[:], ACT.Copy,
                            scale=wtok[:, t * nsub + s:t * nsub + s + 1])
                        nc.sync.dma_start(
                            out=y_d[bass.ts(t * nsub + s, 128), bass.ts(n, 512)],
                            in_=yt[:, bass.ts(n, 512)])

    nc.compile()
    return nc


def _get_pass1():
    if "p1" not in _nc_cache:
        _nc_cache["p1"] = _build_pass1()
    return _nc_cache["p1"]


def _get_pass2(C):
    key = ("p2", C)
    if key not in _nc_cache:
        _nc_cache[key] = _build_pass2(C)
    return _nc_cache[key]


def run(inputs, trace=False, trace_cores=None):
    x = np.ascontiguousarray(np.asarray(inputs["x"], dtype=np.float32))
    gate_w = np.ascontiguousarray(np.asarray(inputs["gate_w"], dtype=np.float32))
    ew1 = np.ascontiguousarray(np.asarray(inputs["expert_w1"], dtype=np.float32))
    eb1 = np.ascontiguousarray(np.asarray(inputs["expert_b1"], dtype=np.float32))
    ew2 = np.ascontiguousarray(np.asarray(inputs["expert_w2"], dtype=np.float32))
    eb2 = np.ascontiguousarray(np.asarray(inputs["expert_b2"], dtype=np.float32))

    xf = x.reshape(N, D)
    xT = np.ascontiguousarray(xf.T)                       # [D, N]
    gwt = np.ascontiguousarray(
        gate_w.reshape(DCH, 128, E).transpose(1, 0, 2).reshape(128, DCH * E))

    perf = {}
    kw = dict(trace=trace)
    if trace and trace_cores is not None:
        kw["trace_cores"] = trace_cores

    # ---- pass 1: router ----
    nc1 = _get_pass1()
    in1 = [{"xt": np.ascontiguousarray(xT[:, c * TPC:(c + 1) * TPC]), "gwt": gwt}
           for c in range(NCORES)]
    r1 = run_bass_kernel_spmd(nc1, in1, core_ids=list(range(NCORES)), **kw)
    perf["pass1_ns"] = r1.exec_time_ns
    wfull = np.concatenate([r1.results[c]["wfull"] for c in range(NCORES)], axis=0)
    auxp = np.concatenate([r1.results[c]["auxp"] for c in range(NCORES)], axis=0)

    # ---- host dispatch (data movement only) ----
    idx = [np.nonzero(wfull[:, e])[0] for e in range(E)]
    maxc = max(len(i) for i in idx)
    C = max(TBLK, ((maxc + TBLK - 1) // TBLK) * TBLK)
    in2 = []
    for e in range(E):
        ie = idx[e]
        xg = np.zeros((D, C), np.float32)
        xg[:, :len(ie)] = xT[:, ie]
        wt = np.zeros(C, np.float32)
        wt[:len(ie)] = wfull[ie, e]
        in2.append({
            "xgt": xg,
            "w1": ew1[e],
            "w2": ew2[e],
            "b1t": np.ascontiguousarray(eb1[e].reshape(FCH, 128).T),
            "b2b": np.ascontiguousarray(np.broadcast_to(eb2[e], (128, D))),
            "wtok": np.ascontiguousarray(wt.reshape(C // 128, 128).T),
            "auxp": auxp,
        })

    # ---- pass 2: expert FFN ----
    nc2 = _get_pass2(C)
    r2 = run_bass_kernel_spmd(nc2, in2, core_ids=list(range(NCORES)), **kw)
    perf["pass2_ns"] = r2.exec_time_ns
    perf["C"] = C
    perf["r1"] = r1
    perf["r2"] = r2

    # ---- host combine (scatter-add of the two expert contributions) ----
    out = np.zeros((N, D), np.float32)
    for e in range(E):
        ie = idx[e]
        out[ie] += r2.results[e]["y"][:len(ie)]
    aux = np.float32(r2.results[0]["aux"][0, 0])
    return out.reshape(B, T, D), aux, perf


def kernel(**inputs):
    out, aux, _ = run(inputs, trace=bool(int(os.environ.get("KERNEL_TRACE", "0"))))
    return out, aux


# revision 11
# speedup vs baseline: 1.0201x; 1.0201x over previous
"""MoE FFN (top-2 routing) Trainium2 kernel.

Strategy (8 NeuronCores, SPMD via run_bass_kernel_spmd):
  Pass 1 (router, data-parallel over tokens): each core takes N/8 = 512
    tokens (x pre-transposed to [D, 512] by the host), computes logits =
    x @ gate_w in fp32 on the PE (gate stationary, tokens moving, then a
    PE transpose back to token-partition layout), derives the top-2
    expert mask and softmax weights on-device with batched DVE ops, and
    emits:
      - wfull [512, E]: per-token router weight for every expert
        (nonzero exactly at the token's top-2 experts)
      - aux partials [1, 17]: softmax-prob column sums (8), top-1
        one-hot column sums (8), sum of logits^2 (1)
  Host dispatch ("all-to-all"): tokens are gathered per expert from the
    nonzero pattern of wfull, laid out transposed ([D, C], zero-padded
    to capacity C), and shipped to the expert's core. Pure data
    movement - no arithmetic on the host.
  Pass 2 (expert FFN, expert-parallel): core e holds expert e's w1/w2
    resident in SBUF and streams its gathered tokens through
      hT = gelu(w1.T @ xT + b1)   [F on partitions]
      y  = wtok * (hT.T @ w2 + b2) [tokens on partitions]
    with float32r matmuls (fp32 data rounded to 11-bit mantissa, 1
    cycle/row on the PE for free dim >= 256). Weights are split into
    per-chunk tiles and DMA-ordered so the PE starts as soon as the
    first chunks land. Also combines the pass-1 aux partials into the
    scalar aux loss on-device.
  Host combine: scatter-add of the (already router-weighted) per-expert
    outputs back to [B, T, D]. Each token receives exactly its two
    expert contributions.
"""

import os
import numpy as np

import concourse.bass as bass
import concourse.mybir as mybir
import concourse.tile as tile
from concourse import bacc
from concourse.alu_op_type import AluOpType
from concourse.bass_utils import run_bass_kernel_spmd
from concourse.masks import make_identity

f32 = mybir.dt.float32
f32r = mybir.dt.float32r
AX = mybir.AxisListType
ACT = mybir.ActivationFunctionType
TT = AluOpType

B, T, D, E, F = 2, 2048, 1024, 8, 2048
N = B * T           # 4096 tokens
NCORES = 8
TPC = N // NCORES   # 512 tokens per core in pass 1
DCH = D // 128      # 8 d-chunks
FCH = F // 128      # 16 f-chunks
TBLK = 384          # pass-2 token block (moving dim; >=256 keeps f32r fast)

Z_LOSS_COEF = 1e-3

_nc_cache = {}


def _build_pass1():
    """Router kernel: one core's 512-token shard."""
    nc = bacc.Bacc("TRN2", target_bir_lowering=False, debug=False,
                   num_devices=NCORES)
    xt_d = nc.dram_tensor("xt", [D, TPC], f32, kind="ExternalInput").ap()
    gwt_d = nc.dram_tensor("gwt", [128, DCH * E], f32, kind="ExternalInput").ap()
    wfull_d = nc.dram_tensor("wfull", [TPC, E], f32, kind="ExternalOutput").ap()
    auxp_d = nc.dram_tensor("auxp", [1, 17], f32, kind="ExternalOutput").ap()

    NB = TPC // 128  # 4 token blocks per core

    with tile.TileContext(nc) as tc:
        with tc.tile_pool(name="sb", bufs=1) as sb, \
             tc.tile_pool(name="ps", bufs=2, space="PSUM") as ps, \
             tc.tile_pool(name="pl", bufs=1, space="PSUM") as pl:
            gw = sb.tile([128, DCH, E], f32)
            nc.gpsimd.dma_start(out=gw[:], in_=gwt_d.rearrange("p (c e) -> p c e", e=E))
            xts = []
            qs = [nc.sync, nc.scalar]
            for d in range(DCH):
                xtd = sb.tile([128, TPC], f32, name=f"xt{d}", tag=f"xt{d}")
                qs[d % 2].dma_start(out=xtd[:], in_=xt_d[bass.ts(d, 128), :])
                xts.append(xtd)
            ident = sb.tile([128, 128], f32)
            make_identity(nc, ident[:])
            ones = sb.tile([128, 1], f32)
            nc.vector.memset(ones[:], 1.0)

            # logits^T [E, TPC] on PSUM (gate stationary, tokens moving, fp32)
            lgT = pl.tile([E, TPC], f32, tag="lgT")
            for d in range(DCH):
                nc.tensor.matmul(lgT[:], lhsT=gw[:, d, :], rhs=xts[d][:],
                                 start=(d == 0), stop=(d == DCH - 1))
            lgTs = sb.tile([E, TPC], f32)
            nc.scalar.copy(lgTs[:], lgT[:])
            # transpose back to [128 tokens, E] per block, gather into SBUF
            lgs = sb.tile([128, NB, E], f32)
            for b in range(NB):
                lgp = ps.tile([128, E], f32, tag="lgp")
                nc.tensor.transpose(lgp[:], lgTs[:, bass.ts(b, 128)],
                                    ident[:E, :E])
                nc.scalar.copy(lgs[:, b, :], lgp[:])

            def bc(ap):  # [128, NB] -> [128, NB, E] stride-0 broadcast
                return ap[:, :, None].broadcast_to([128, NB, E])

            acc = sb.tile([128, 17], f32)   # [probs 8 | onehot 8 | z 1]
            t1 = sb.tile([128, NB], f32)
            nc.vector.tensor_reduce(t1[:], lgs[:], axis=AX.X, op=TT.max)
            eq1 = sb.tile([128, NB, E], f32)
            nc.vector.tensor_tensor(eq1[:], lgs[:], bc(t1), TT.is_equal)
            msk = sb.tile([128, NB, E], f32)
            nc.vector.scalar_tensor_tensor(out=msk[:], in0=eq1[:], scalar=-1e30,
                                           in1=lgs[:], op0=TT.mult, op1=TT.add)
            t2 = sb.tile([128, NB], f32)
            nc.vector.tensor_reduce(t2[:], msk[:], axis=AX.X, op=TT.max)
            eq2 = sb.tile([128, NB, E], f32)
            nc.vector.tensor_tensor(eq2[:], msk[:], bc(t2), TT.is_equal)
            # top-2 softmax weights: wa = 1/(1+exp(t2-t1)), wb = 1-wa
            d21 = sb.tile([128, NB], f32)
            nc.vector.tensor_sub(d21[:], t2[:], t1[:])
            ex = sb.tile([128, NB], f32)
            nc.scalar.activation(ex[:], d21[:], ACT.Exp)
            den = sb.tile([128, NB], f32)
            nc.vector.tensor_scalar_add(den[:], ex[:], 1.0)
            wa = sb.tile([128, NB], f32)
            nc.vector.reciprocal(wa[:], den[:])
            wb = sb.tile([128, NB], f32)
            nc.vector.tensor_mul(wb[:], ex[:], wa[:])
            # wfull = wa*eq1 + wb*eq2
            wf = sb.tile([128, NB, E], f32)
            nc.vector.tensor_tensor(wf[:], eq1[:], bc(wa), TT.mult)
            wf2 = sb.tile([128, NB, E], f32)
            nc.vector.tensor_tensor(wf2[:], eq2[:], bc(wb), TT.mult)
            nc.vector.tensor_add(wf[:], wf[:], wf2[:])
            nc.sync.dma_start(out=wfull_d.rearrange("(b p) e -> p b e", p=128),
                              in_=wf[:])
            # softmax probs for aux
            sub = sb.tile([128, NB, E], f32)
            nc.vector.tensor_tensor(sub[:], lgs[:], bc(t1), TT.subtract)
            pr = sb.tile([128, NB, E], f32)
            nc.scalar.activation(pr[:], sub[:], ACT.Exp)
            se = sb.tile([128, NB], f32)
            nc.vector.tensor_reduce(se[:], pr[:], axis=AX.X, op=TT.add)
            rs = sb.tile([128, NB], f32)
            nc.vector.reciprocal(rs[:], se[:])
            prn = sb.tile([128, NB, E], f32)
            nc.vector.tensor_tensor(prn[:], pr[:], bc(rs), TT.mult)
            # block-sums into acc
            nc.vector.tensor_add(acc[:, 0:E], prn[:, 0, :], prn[:, 1, :])
            nc.vector.tensor_add(acc[:, 0:E], acc[:, 0:E], prn[:, 2, :])
            nc.vector.tensor_add(acc[:, 0:E], acc[:, 0:E], prn[:, 3, :])
            nc.vector.tensor_add(acc[:, E:2 * E], eq1[:, 0, :], eq1[:, 1, :])
            nc.vector.tensor_add(acc[:, E:2 * E], acc[:, E:2 * E], eq1[:, 2, :])
            nc.vector.tensor_add(acc[:, E:2 * E], acc[:, E:2 * E], eq1[:, 3, :])
            sq = sb.tile([128, NB, E], f32)
            nc.scalar.activation(sq[:], lgs[:], ACT.Square,
                                 accum_out=acc[:, 16:17])
            # column sums over the 128 partitions via ones-matmul
            accp = ps.tile([1, 17], f32, tag="accp")
            nc.tensor.matmul(accp[:], lhsT=ones[:], rhs=acc[:], start=True,
                             stop=True)
            accs = sb.tile([1, 17], f32)
            nc.vector.tensor_copy(accs[:], accp[:])
            nc.sync.dma_start(out=auxp_d[:], in_=accs[:])

    nc.compile()
    return nc


def _build_pass2(C, has_b2=True):
    """Expert FFN kernel: one expert's C gathered tokens (C % TBLK == 0)."""
    nc = bacc.Bacc("TRN2", target_bir_lowering=False, debug=False,
                   num_devices=NCORES)
    xgt_d = nc.dram_tensor("xgt", [D, C], f32r, kind="ExternalInput").ap()
    w1_d = nc.dram_tensor("w1", [D, F], f32r, kind="ExternalInput").ap()
    w2_d = nc.dram_tensor("w2", [F, D], f32r, kind="ExternalInput").ap()
    b1t_d = nc.dram_tensor("b1t", [128, FCH], f32, kind="ExternalInput").ap()
    b2b_d = nc.dram_tensor("b2b", [128, D], f32, kind="ExternalInput").ap()
    wtok_d = nc.dram_tensor("wtok", [128, C // 128], f32, kind="ExternalInput").ap()
    auxp_d = nc.dram_tensor("auxp", [NCORES, 17], f32, kind="ExternalInput").ap()
    y_d = nc.dram_tensor("y", [C, D], f32, kind="ExternalOutput").ap()
    aux_d = nc.dram_tensor("aux", [1, 1], f32, kind="ExternalOutput").ap()

    nblk = C // TBLK
    nsub = TBLK // 128

    with tile.TileContext(nc) as tc:
        with tc.tile_pool(name="wt", bufs=1) as wt, \
             tc.tile_pool(name="xg", bufs=2) as xg, \
             tc.tile_pool(name="hp", bufs=1) as hp, \
             tc.tile_pool(name="yo", bufs=2) as yo, \
             tc.tile_pool(name="sm", bufs=1) as sm, \
             tc.tile_pool(name="ph", bufs=6, space="PSUM") as ph, \
             tc.tile_pool(name="py", bufs=2, space="PSUM") as py:

            def xgt_block(t):
                xts = []
                for d in range(DCH):
                    xtd = xg.tile([128, TBLK], f32r, name=f"xt{t}_{d}",
                                  tag=f"xtd{d}")
                    nc.sync.dma_start(
                        out=xtd[:],
                        in_=xgt_d[bass.ts(d, 128), bass.ts(t, TBLK)])
                    xts.append(xtd)
                return xts

            # DMA issue order: first block's tokens + w1 first (mm1 needs
            # them), then small constants, then w2 (mm2 starts ~25us in).
            xts0 = xgt_block(0)
            w1c = []
            for d in range(DCH):
                w1d = wt.tile([128, F], f32r, name=f"w1_{d}")
                nc.sync.dma_start(out=w1d[:], in_=w1_d[bass.ts(d, 128), :])
                w1c.append(w1d)
            b1t = wt.tile([128, FCH], f32)
            nc.sync.dma_start(out=b1t[:], in_=b1t_d[:])
            wtok = wt.tile([128, C // 128], f32)
            nc.sync.dma_start(out=wtok[:], in_=wtok_d[:])
            w2c = []
            for f in range(FCH):
                w2f = wt.tile([128, D], f32r, name=f"w2_{f}")
                nc.sync.dma_start(out=w2f[:], in_=w2_d[bass.ts(f, 128), :])
                w2c.append(w2f)
            if has_b2:
                b2b = wt.tile([128, D], f32)
                nc.gpsimd.dma_start(out=b2b[:], in_=b2b_d[:])

            # aux combine (tiny, once)
            auxp = sm.tile([NCORES, 17], f32)
            nc.gpsimd.dma_start(out=auxp[:], in_=auxp_d[:])
            ones8 = sm.tile([NCORES, 1], f32)
            nc.vector.memset(ones8[:], 1.0)
            auxs = sm.tile([1, 17], f32)
            auxt = py.tile([1, 17], f32, tag="yps", name="auxt")
            nc.tensor.matmul(auxt[:], lhsT=ones8[:], rhs=auxp[:], start=True,
                             stop=True)
            nc.vector.tensor_copy(auxs[:], auxt[:])
            prod = sm.tile([1, E], f32)
            nc.vector.tensor_mul(prod[:], auxs[:, 0:E], auxs[:, E:2 * E])
            psum_ = sm.tile([1, 1], f32)
            nc.vector.tensor_reduce(psum_[:], prod[:], axis=AX.X, op=TT.add)
            zt = sm.tile([1, 1], f32)
            nc.vector.tensor_scalar_mul(zt[:], auxs[:, 16:17],
                                        float(Z_LOSS_COEF / (N * E)))
            auxo = sm.tile([1, 1], f32)
            nc.vector.scalar_tensor_tensor(out=auxo[:], in0=psum_[:],
                                           scalar=float(E) / (float(N) * float(N)),
                                           in1=zt[:], op0=TT.mult, op1=TT.add)
            nc.gpsimd.dma_start(out=aux_d[:], in_=auxo[:])

            # main FFN loop. mm1 runs d-outer over f-groups of <=6 so the
            # PE consumes w1/xgt chunks in DMA-arrival order on block 0.
            FG = [list(range(0, 6)), list(range(6, 12)), list(range(12, 16))]
            for t in range(nblk):
                xts = xts0 if t == 0 else xgt_block(t)
                hT = hp.tile([128, FCH, TBLK], f32r, tag="hT")
                for fg in FG:
                    hps_l = [ph.tile([128, TBLK], f32, tag="hps",
                                     name=f"hps{t}_{f}") for f in fg]
                    for d in range(DCH):
                        for j, f in enumerate(fg):
                            nc.tensor.matmul(hps_l[j][:],
                                             lhsT=w1c[d][:, bass.ts(f, 128)],
                                             rhs=xts[d][:],
                                             start=(d == 0), stop=(d == DCH - 1))
                    # hT = gelu(w1.T x + b1), erf flavor
                    for j, f in enumerate(fg):
                        nc.scalar.activation(hT[:, f, :], hps_l[j][:], ACT.Gelu,
                                             bias=b1t[:, f:f + 1])
                for s in range(nsub):
                    yt = yo.tile([128, D], f32, tag="yt")
                    for n in range(2):
                        yps = py.tile([128, 512], f32, tag="yps")
                        for f in range(FCH):
                            nc.tensor.matmul(yps[:],
                                             lhsT=hT[:, f, bass.ts(s, 128)],
                                             rhs=w2c[f][:, bass.ts(n, 512)],
                                             start=(f == 0), stop=(f == FCH - 1))
                        # y = wtok * (psum + b2)
                        if has_b2:
                            tb = yo.tile([128, 512], f32, tag="tb")
                            nc.vector.tensor_add(tb[:], yps[:],
                                                 b2b[:, bass.ts(n, 512)])
                            src_ap = tb[:]
                        else:
                            src_ap = yps[:]
                        nc.scalar.activation(
                            yt[:, bass.ts(n, 512)], src_ap, ACT.Copy,
                            scale=wtok[:, t * nsub + s:t * nsub + s + 1])
                        nc.gpsimd.dma_start(
                            out=y_d[bass.ts(t * nsub + s, 128), bass.ts(n, 512)],
                            in_=yt[:, bass.ts(n, 512)])

    nc.compile()
    return nc


def _get_pass1():
    if "p1" not in _nc_cache:
        _nc_cache["p1"] = _build_pass1()
    return _nc_cache["p1"]


def _get_pass2(C, has_b2=True):
    key = ("p2", C, has_b2)
    if key not in _nc_cache:
        _nc_cache[key] = _build_pass2(C, has_b2)
    return _nc_cache[key]


def run(inputs, trace=False, trace_cores=None):
    x = np.ascontiguousarray(np.asarray(inputs["x"], dtype=np.float32))
    gate_w = np.ascontiguousarray(np.asarray(inputs["gate_w"], dtype=np.float32))
    ew1 = np.ascontiguousarray(np.asarray(inputs["expert_w1"], dtype=np.float32))
    eb1 = np.ascontiguousarray(np.asarray(inputs["expert_b1"], dtype=np.float32))
    ew2 = np.ascontiguousarray(np.asarray(inputs["expert_w2"], dtype=np.float32))
    eb2 = np.ascontiguousarray(np.asarray(inputs["expert_b2"], dtype=np.float32))

    xf = x.reshape(N, D)
    xT = np.ascontiguousarray(xf.T)                       # [D, N]
    gwt = np.ascontiguousarray(
        gate_w.reshape(DCH, 128, E).transpose(1, 0, 2).reshape(128, DCH * E))

    perf = {}
    kw = dict(trace=trace)
    if trace and trace_cores is not None:
        kw["trace_cores"] = trace_cores

    # ---- pass 1: router ----
    nc1 = _get_pass1()
    in1 = [{"xt": np.ascontiguousarray(xT[:, c * TPC:(c + 1) * TPC]), "gwt": gwt}
           for c in range(NCORES)]
    r1 = run_bass_kernel_spmd(nc1, in1, core_ids=list(range(NCORES)), **kw)
    perf["pass1_ns"] = r1.exec_time_ns
    wfull = np.concatenate([r1.results[c]["wfull"] for c in range(NCORES)], axis=0)
    auxp = np.concatenate([r1.results[c]["auxp"] for c in range(NCORES)], axis=0)

    # ---- host dispatch (data movement only) ----
    idx = [np.nonzero(wfull[:, e])[0] for e in range(E)]
    maxc = max(len(i) for i in idx)
    C = max(TBLK, ((maxc + TBLK - 1) // TBLK) * TBLK)
    in2 = []
    for e in range(E):
        ie = idx[e]
        xg = np.zeros((D, C), np.float32)
        xg[:, :len(ie)] = xT[:, ie]
        wt = np.zeros(C, np.float32)
        wt[:len(ie)] = wfull[ie, e]
        in2.append({
            "xgt": xg,
            "w1": ew1[e],
            "w2": ew2[e],
            "b1t": np.ascontiguousarray(eb1[e].reshape(FCH, 128).T),
            "b2b": np.ascontiguousarray(np.broadcast_to(eb2[e], (128, D))),
            "wtok": np.ascontiguousarray(wt.reshape(C // 128, 128).T),
            "auxp": auxp,
        })

    # ---- pass 2: expert FFN ----
    has_b2 = bool(np.any(eb2))
    nc2 = _get_pass2(C, has_b2)
    r2 = run_bass_kernel_spmd(nc2, in2, core_ids=list(range(NCORES)), **kw)
    perf["pass2_ns"] = r2.exec_time_ns
    perf["C"] = C
    perf["r1"] = r1
    perf["r2"] = r2

    # ---- host combine (scatter-add of the two expert contributions) ----
    out = np.zeros((N, D), np.float32)
    for e in range(E):
        ie = idx[e]
        out[ie] += r2.results[e]["y"][:len(ie)]
    aux = np.float32(r2.results[0]["aux"][0, 0])
    return out.reshape(B, T, D), aux, perf


def kernel(**inputs):
    out, aux, _ = run(inputs, trace=bool(int(os.environ.get("KERNEL_TRACE", "0"))))
    return out, aux


# revision 12
# speedup vs baseline: 1.0285x; 1.0082x over previous
"""MoE FFN (top-2 routing) Trainium2 kernel.

Strategy (8 NeuronCores, SPMD via run_bass_kernel_spmd):
  Pass 1 (router, data-parallel over tokens): each core takes N/8 = 512
    tokens (x pre-transposed to [D, 512] by the host), computes logits =
    x @ gate_w in fp32 on the PE (gate stationary, tokens moving, then a
    PE transpose back to token-partition layout), derives the top-2
    expert mask and softmax weights on-device with batched DVE ops, and
    emits:
      - wfull [512, E]: per-token router weight for every expert
        (nonzero exactly at the token's top-2 experts)
      - aux partials [1, 17]: softmax-prob column sums (8), top-1
        one-hot column sums (8), sum of logits^2 (1)
  Host dispatch ("all-to-all"): tokens are gathered per expert from the
    nonzero pattern of wfull, laid out transposed ([D, C], zero-padded
    to capacity C), and shipped to the expert's core. Pure data
    movement - no arithmetic on the host.
  Pass 2 (expert FFN, expert-parallel): core e holds expert e's w1/w2
    resident in SBUF and streams its gathered tokens through
      hT = gelu(w1.T @ xT + b1)   [F on partitions]
      y  = wtok * (hT.T @ w2 + b2) [tokens on partitions]
    with float32r matmuls (fp32 data rounded to 11-bit mantissa, 1
    cycle/row on the PE for free dim >= 256). Weights are split into
    per-chunk tiles and DMA-ordered so the PE starts as soon as the
    first chunks land. Also combines the pass-1 aux partials into the
    scalar aux loss on-device.
  Host combine: scatter-add of the (already router-weighted) per-expert
    outputs back to [B, T, D]. Each token receives exactly its two
    expert contributions.
"""

import os
import numpy as np

import concourse.bass as bass
import concourse.mybir as mybir
import concourse.tile as tile
from concourse import bacc
from concourse.alu_op_type import AluOpType
from concourse.bass_utils import run_bass_kernel_spmd
from concourse.masks import make_identity

f32 = mybir.dt.float32
f32r = mybir.dt.float32r
AX = mybir.AxisListType
ACT = mybir.ActivationFunctionType
TT = AluOpType

B, T, D, E, F = 2, 2048, 1024, 8, 2048
N = B * T           # 4096 tokens
NCORES = 8
TPC = N // NCORES   # 512 tokens per core in pass 1
DCH = D // 128      # 8 d-chunks
FCH = F // 128      # 16 f-chunks
TBLK = 384          # pass-2 token block (moving dim; >=256 keeps f32r fast)

Z_LOSS_COEF = 1e-3

_nc_cache = {}


def _build_pass1():
    """Router kernel: one core's 512-token shard."""
    nc = bacc.Bacc("TRN2", target_bir_lowering=False, debug=False,
                   num_devices=NCORES)
    xt_d = nc.dram_tensor("xt", [D, TPC], f32, kind="ExternalInput").ap()
    gwt_d = nc.dram_tensor("gwt", [128, DCH * E], f32, kind="ExternalInput").ap()
    wfull_d = nc.dram_tensor("wfull", [TPC, E], f32, kind="ExternalOutput").ap()
    auxp_d = nc.dram_tensor("auxp", [1, 17], f32, kind="ExternalOutput").ap()

    NB = TPC // 128  # 4 token blocks per core

    with tile.TileContext(nc) as tc:
        with tc.tile_pool(name="sb", bufs=1) as sb, \
             tc.tile_pool(name="ps", bufs=2, space="PSUM") as ps, \
             tc.tile_pool(name="pl", bufs=1, space="PSUM") as pl:
            gw = sb.tile([128, DCH, E], f32)
            nc.gpsimd.dma_start(out=gw[:], in_=gwt_d.rearrange("p (c e) -> p c e", e=E))
            xts = []
            qs = [nc.sync, nc.scalar]
            for d in range(DCH):
                xtd = sb.tile([128, TPC], f32, name=f"xt{d}", tag=f"xt{d}")
                qs[d % 2].dma_start(out=xtd[:], in_=xt_d[bass.ts(d, 128), :])
                xts.append(xtd)
            ident = sb.tile([128, 128], f32)
            make_identity(nc, ident[:])
            ones = sb.tile([128, 1], f32)
            nc.vector.memset(ones[:], 1.0)
            # PE warm-up during the DMA window (HAM needs ~3.4us of
            # activity to unthrottle 1.2 -> 2.4 GHz)
            wsc = sb.tile([128, 64], f32)
            nc.vector.memset(wsc[:], 1.0)
            wps = ps.tile([64, 64], f32, tag="lgp", name="wps")
            for _ in range(36):
                nc.tensor.matmul(wps[:], lhsT=wsc[:, :64], rhs=wsc[:, :64],
                                 start=True, stop=True)

            # logits^T [E, TPC] on PSUM (gate stationary, tokens moving, fp32)
            lgT = pl.tile([E, TPC], f32, tag="lgT")
            for d in range(DCH):
                nc.tensor.matmul(lgT[:], lhsT=gw[:, d, :], rhs=xts[d][:],
                                 start=(d == 0), stop=(d == DCH - 1))
            lgTs = sb.tile([E, TPC], f32)
            nc.scalar.copy(lgTs[:], lgT[:])
            # transpose back to [128 tokens, E] per block, gather into SBUF
            lgs = sb.tile([128, NB, E], f32)
            for b in range(NB):
                lgp = ps.tile([128, E], f32, tag="lgp")
                nc.tensor.transpose(lgp[:], lgTs[:, bass.ts(b, 128)],
                                    ident[:E, :E])
                nc.scalar.copy(lgs[:, b, :], lgp[:])

            def bc(ap):  # [128, NB] -> [128, NB, E] stride-0 broadcast
                return ap[:, :, None].broadcast_to([128, NB, E])

            acc = sb.tile([128, 17], f32)   # [probs 8 | onehot 8 | z 1]
            t1 = sb.tile([128, NB], f32)
            nc.vector.tensor_reduce(t1[:], lgs[:], axis=AX.X, op=TT.max)
            eq1 = sb.tile([128, NB, E], f32)
            nc.vector.tensor_tensor(eq1[:], lgs[:], bc(t1), TT.is_equal)
            msk = sb.tile([128, NB, E], f32)
            nc.vector.scalar_tensor_tensor(out=msk[:], in0=eq1[:], scalar=-1e30,
                                           in1=lgs[:], op0=TT.mult, op1=TT.add)
            t2 = sb.tile([128, NB], f32)
            nc.vector.tensor_reduce(t2[:], msk[:], axis=AX.X, op=TT.max)
            eq2 = sb.tile([128, NB, E], f32)
            nc.vector.tensor_tensor(eq2[:], msk[:], bc(t2), TT.is_equal)
            # top-2 softmax weights: wa = 1/(1+exp(t2-t1)), wb = 1-wa
            d21 = sb.tile([128, NB], f32)
            nc.vector.tensor_sub(d21[:], t2[:], t1[:])
            ex = sb.tile([128, NB], f32)
            nc.scalar.activation(ex[:], d21[:], ACT.Exp)
            den = sb.tile([128, NB], f32)
            nc.vector.tensor_scalar_add(den[:], ex[:], 1.0)
            wa = sb.tile([128, NB], f32)
            nc.vector.reciprocal(wa[:], den[:])
            wb = sb.tile([128, NB], f32)
            nc.vector.tensor_mul(wb[:], ex[:], wa[:])
            # wfull = wa*eq1 + wb*eq2
            wf = sb.tile([128, NB, E], f32)
            nc.vector.tensor_tensor(wf[:], eq1[:], bc(wa), TT.mult)
            wf2 = sb.tile([128, NB, E], f32)
            nc.vector.tensor_tensor(wf2[:], eq2[:], bc(wb), TT.mult)
            nc.vector.tensor_add(wf[:], wf[:], wf2[:])
            nc.sync.dma_start(out=wfull_d.rearrange("(b p) e -> p b e", p=128),
                              in_=wf[:])
            # softmax probs for aux
            sub = sb.tile([128, NB, E], f32)
            nc.vector.tensor_tensor(sub[:], lgs[:], bc(t1), TT.subtract)
            pr = sb.tile([128, NB, E], f32)
            nc.scalar.activation(pr[:], sub[:], ACT.Exp)
            se = sb.tile([128, NB], f32)
            nc.vector.tensor_reduce(se[:], pr[:], axis=AX.X, op=TT.add)
            rs = sb.tile([128, NB], f32)
            nc.vector.reciprocal(rs[:], se[:])
            prn = sb.tile([128, NB, E], f32)
            nc.vector.tensor_tensor(prn[:], pr[:], bc(rs), TT.mult)
            # block-sums into acc
            nc.vector.tensor_add(acc[:, 0:E], prn[:, 0, :], prn[:, 1, :])
            nc.vector.tensor_add(acc[:, 0:E], acc[:, 0:E], prn[:, 2, :])
            nc.vector.tensor_add(acc[:, 0:E], acc[:, 0:E], prn[:, 3, :])
            nc.vector.tensor_add(acc[:, E:2 * E], eq1[:, 0, :], eq1[:, 1, :])
            nc.vector.tensor_add(acc[:, E:2 * E], acc[:, E:2 * E], eq1[:, 2, :])
            nc.vector.tensor_add(acc[:, E:2 * E], acc[:, E:2 * E], eq1[:, 3, :])
            sq = sb.tile([128, NB, E], f32)
            nc.scalar.activation(sq[:], lgs[:], ACT.Square,
                                 accum_out=acc[:, 16:17])
            # column sums over the 128 partitions via ones-matmul
            accp = ps.tile([1, 17], f32, tag="accp")
            nc.tensor.matmul(accp[:], lhsT=ones[:], rhs=acc[:], start=True,
                             stop=True)
            accs = sb.tile([1, 17], f32)
            nc.vector.tensor_copy(accs[:], accp[:])
            nc.sync.dma_start(out=auxp_d[:], in_=accs[:])

    nc.compile()
    return nc


def _build_pass2(C, has_b2=True):
    """Expert FFN kernel: one expert's C gathered tokens (C % TBLK == 0)."""
    nc = bacc.Bacc("TRN2", target_bir_lowering=False, debug=False,
                   num_devices=NCORES)
    xgt_d = nc.dram_tensor("xgt", [D, C], f32r, kind="ExternalInput").ap()
    w1_d = nc.dram_tensor("w1", [D, F], f32r, kind="ExternalInput").ap()
    w2_d = nc.dram_tensor("w2", [F, D], f32r, kind="ExternalInput").ap()
    b1t_d = nc.dram_tensor("b1t", [128, FCH], f32, kind="ExternalInput").ap()
    b2b_d = nc.dram_tensor("b2b", [128, D], f32, kind="ExternalInput").ap()
    wtok_d = nc.dram_tensor("wtok", [128, C // 128], f32, kind="ExternalInput").ap()
    auxp_d = nc.dram_tensor("auxp", [NCORES, 17], f32, kind="ExternalInput").ap()
    y_d = nc.dram_tensor("y", [C, D], f32, kind="ExternalOutput").ap()
    aux_d = nc.dram_tensor("aux", [1, 1], f32, kind="ExternalOutput").ap()

    nblk = C // TBLK
    nsub = TBLK // 128

    with tile.TileContext(nc) as tc:
        with tc.tile_pool(name="wt", bufs=1) as wt, \
             tc.tile_pool(name="xg", bufs=2) as xg, \
             tc.tile_pool(name="hp", bufs=1) as hp, \
             tc.tile_pool(name="yo", bufs=2) as yo, \
             tc.tile_pool(name="sm", bufs=1) as sm, \
             tc.tile_pool(name="ph", bufs=6, space="PSUM") as ph, \
             tc.tile_pool(name="py", bufs=2, space="PSUM") as py:

            def xgt_block(t):
                xts = []
                for d in range(DCH):
                    xtd = xg.tile([128, TBLK], f32r, name=f"xt{t}_{d}",
                                  tag=f"xtd{d}")
                    nc.sync.dma_start(
                        out=xtd[:],
                        in_=xgt_d[bass.ts(d, 128), bass.ts(t, TBLK)])
                    xts.append(xtd)
                return xts

            # DMA issue order: first block's tokens + w1 first (mm1 needs
            # them), then small constants, then w2 (mm2 starts ~25us in).
            xts0 = xgt_block(0)
            w1c = []
            for d in range(DCH):
                w1d = wt.tile([128, F], f32r, name=f"w1_{d}")
                nc.sync.dma_start(out=w1d[:], in_=w1_d[bass.ts(d, 128), :])
                w1c.append(w1d)
            b1t = wt.tile([128, FCH], f32)
            nc.sync.dma_start(out=b1t[:], in_=b1t_d[:])
            wtok = wt.tile([128, C // 128], f32)
            nc.sync.dma_start(out=wtok[:], in_=wtok_d[:])
            w2c = []
            for f in range(FCH):
                w2f = wt.tile([128, D], f32r, name=f"w2_{f}")
                nc.sync.dma_start(out=w2f[:], in_=w2_d[bass.ts(f, 128), :])
                w2c.append(w2f)
            if has_b2:
                b2b = wt.tile([128, D], f32)
                nc.gpsimd.dma_start(out=b2b[:], in_=b2b_d[:])

            # PE warm-up during the weight-stream window
            wsc = sm.tile([128, 64], f32)
            nc.vector.memset(wsc[:], 1.0)
            wps = py.tile([64, 64], f32, tag="yps", name="wps")
            for _ in range(36):
                nc.tensor.matmul(wps[:], lhsT=wsc[:, :64], rhs=wsc[:, :64],
                                 start=True, stop=True)

            # aux combine (tiny, once)
            auxp = sm.tile([NCORES, 17], f32)
            nc.gpsimd.dma_start(out=auxp[:], in_=auxp_d[:])
            ones8 = sm.tile([NCORES, 1], f32)
            nc.vector.memset(ones8[:], 1.0)
            auxs = sm.tile([1, 17], f32)
            auxt = py.tile([1, 17], f32, tag="yps", name="auxt")
            nc.tensor.matmul(auxt[:], lhsT=ones8[:], rhs=auxp[:], start=True,
                             stop=True)
            nc.vector.tensor_copy(auxs[:], auxt[:])
            prod = sm.tile([1, E], f32)
            nc.vector.tensor_mul(prod[:], auxs[:, 0:E], auxs[:, E:2 * E])
            psum_ = sm.tile([1, 1], f32)
            nc.vector.tensor_reduce(psum_[:], prod[:], axis=AX.X, op=TT.add)
            zt = sm.tile([1, 1], f32)
            nc.vector.tensor_scalar_mul(zt[:], auxs[:, 16:17],
                                        float(Z_LOSS_COEF / (N * E)))
            auxo = sm.tile([1, 1], f32)
            nc.vector.scalar_tensor_tensor(out=auxo[:], in0=psum_[:],
                                           scalar=float(E) / (float(N) * float(N)),
                                           in1=zt[:], op0=TT.mult, op1=TT.add)
            nc.gpsimd.dma_start(out=aux_d[:], in_=auxo[:])

            # main FFN loop. mm1 runs d-outer over f-groups of <=6 so the
            # PE consumes w1/xgt chunks in DMA-arrival order on block 0.
            FG = [list(range(0, 6)), list(range(6, 12)), list(range(12, 16))]
            for t in range(nblk):
                xts = xts0 if t == 0 else xgt_block(t)
                hT = hp.tile([128, FCH, TBLK], f32r, tag="hT")
                for fg in FG:
                    hps_l = [ph.tile([128, TBLK], f32, tag="hps",
                                     name=f"hps{t}_{f}") for f in fg]
                    for d in range(DCH):
                        for j, f in enumerate(fg):
                            nc.tensor.matmul(hps_l[j][:],
                                             lhsT=w1c[d][:, bass.ts(f, 128)],
                                             rhs=xts[d][:],
                                             start=(d == 0), stop=(d == DCH - 1))
                    # hT = gelu(w1.T x + b1), erf flavor
                    for j, f in enumerate(fg):
                        nc.scalar.activation(hT[:, f, :], hps_l[j][:], ACT.Gelu,
                                             bias=b1t[:, f:f + 1])
                for s in range(nsub):
                    yt = yo.tile([128, D], f32, tag="yt")
                    for n in range(2):
                        yps = py.tile([128, 512], f32, tag="yps")
                        for f in range(FCH):
                            nc.tensor.matmul(yps[:],
                                             lhsT=hT[:, f, bass.ts(s, 128)],
                                             rhs=w2c[f][:, bass.ts(n, 512)],
                                             start=(f == 0), stop=(f == FCH - 1))
                        # y = wtok * (psum + b2)
                        if has_b2:
                            tb = yo.tile([128, 512], f32, tag="tb")
                            nc.vector.tensor_add(tb[:], yps[:],
                                                 b2b[:, bass.ts(n, 512)])
                            src_ap = tb[:]
                        else:
                            src_ap = yps[:]
                        nc.scalar.activation(
                            yt[:, bass.ts(n, 512)], src_ap, ACT.Copy,
                            scale=wtok[:, t * nsub + s:t * nsub + s + 1])
                        nc.scalar.dma_start(
                            out=y_d[bass.ts(t * nsub + s, 128), bass.ts(n, 512)],
                            in_=yt[:, bass.ts(n, 512)])

    nc.compile()
    return nc


def _get_pass1():
    if "p1" not in _nc_cache:
        _nc_cache["p1"] = _build_pass1()
    return _nc_cache["p1"]


def _get_pass2(C, has_b2=True):
    key = ("p2", C, has_b2)
    if key not in _nc_cache:
        _nc_cache[key] = _build_pass2(C, has_b2)
    return _nc_cache[key]


def run(inputs, trace=False, trace_cores=None):
    x = np.ascontiguousarray(np.asarray(inputs["x"], dtype=np.float32))
    gate_w = np.ascontiguousarray(np.asarray(inputs["gate_w"], dtype=np.float32))
    ew1 = np.ascontiguousarray(np.asarray(inputs["expert_w1"], dtype=np.float32))
    eb1 = np.ascontiguousarray(np.asarray(inputs["expert_b1"], dtype=np.float32))
    ew2 = np.ascontiguousarray(np.asarray(inputs["expert_w2"], dtype=np.float32))
    eb2 = np.ascontiguousarray(np.asarray(inputs["expert_b2"], dtype=np.float32))

    xf = x.reshape(N, D)
    xT = np.ascontiguousarray(xf.T)                       # [D, N]
    gwt = np.ascontiguousarray(
        gate_w.reshape(DCH, 128, E).transpose(1, 0, 2).reshape(128, DCH * E))

    perf = {}
    kw = dict(trace=trace)
    if trace and trace_cores is not None:
        kw["trace_cores"] = trace_cores

    # ---- pass 1: router ----
    nc1 = _get_pass1()
    in1 = [{"xt": np.ascontiguousarray(xT[:, c * TPC:(c + 1) * TPC]), "gwt": gwt}
           for c in range(NCORES)]
    r1 = run_bass_kernel_spmd(nc1, in1, core_ids=list(range(NCORES)), **kw)
    perf["pass1_ns"] = r1.exec_time_ns
    wfull = np.concatenate([r1.results[c]["wfull"] for c in range(NCORES)], axis=0)
    auxp = np.concatenate([r1.results[c]["auxp"] for c in range(NCORES)], axis=0)

    # ---- host dispatch (data movement only) ----
    idx = [np.nonzero(wfull[:, e])[0] for e in range(E)]
    maxc = max(len(i) for i in idx)
    C = max(TBLK, ((maxc + TBLK - 1) // TBLK) * TBLK)
    in2 = []
    for e in range(E):
        ie = idx[e]
        xg = np.zeros((D, C), np.float32)
        xg[:, :len(ie)] = xT[:, ie]
        wt = np.zeros(C, np.float32)
        wt[:len(ie)] = wfull[ie, e]
        in2.append({
            "xgt": xg,
            "w1": ew1[e],
            "w2": ew2[e],
            "b1t": np.ascontiguousarray(eb1[e].reshape(FCH, 128).T),
            "b2b": np.ascontiguousarray(np.broadcast_to(eb2[e], (128, D))),
            "wtok": np.ascontiguousarray(wt.reshape(C // 128, 128).T),
            "auxp": auxp,
        })

    # ---- pass 2: expert FFN ----
    has_b2 = bool(np.any(eb2))
    nc2 = _get_pass2(C, has_b2)
    r2 = run_bass_kernel_spmd(nc2, in2, core_ids=list(range(NCORES)), **kw)
    perf["pass2_ns"] = r2.exec_time_ns
    perf["C"] = C
    perf["r1"] = r1
    perf["r2"] = r2

    # ---- host combine (scatter-add of the two expert contributions) ----
    out = np.zeros((N, D), np.float32)
    for e in range(E):
        ie = idx[e]
        out[ie] += r2.results[e]["y"][:len(ie)]
    aux = np.float32(r2.results[0]["aux"][0, 0])
    return out.reshape(B, T, D), aux, perf


def kernel(**inputs):
    out, aux, _ = run(inputs, trace=bool(int(os.environ.get("KERNEL_TRACE", "0"))))
    return out, aux


# revision 13
# speedup vs baseline: 1.0292x; 1.0007x over previous
"""MoE FFN (top-2 routing) Trainium2 kernel.

Strategy (8 NeuronCores, SPMD via run_bass_kernel_spmd):
  Pass 1 (router, data-parallel over tokens): each core takes N/8 = 512
    tokens (x pre-transposed to [D, 512] by the host), computes logits =
    x @ gate_w in fp32 on the PE (gate stationary, tokens moving, then a
    PE transpose back to token-partition layout), derives the top-2
    expert mask and softmax weights on-device with batched DVE ops, and
    emits:
      - wfull [512, E]: per-token router weight for every expert
        (nonzero exactly at the token's top-2 experts)
      - aux partials [1, 17]: softmax-prob column sums (8), top-1
        one-hot column sums (8), sum of logits^2 (1)
  Host dispatch ("all-to-all"): tokens are gathered per expert from the
    nonzero pattern of wfull, laid out transposed ([D, C], zero-padded
    to capacity C), and shipped to the expert's core. Pure data
    movement - no arithmetic on the host.
  Pass 2 (expert FFN, expert-parallel): core e holds expert e's w1/w2
    resident in SBUF and streams its gathered tokens through
      hT = gelu(w1.T @ xT + b1)   [F on partitions]
      y  = wtok * (hT.T @ w2 + b2) [tokens on partitions]
    with float32r matmuls (fp32 data rounded to 11-bit mantissa, 1
    cycle/row on the PE for free dim >= 256). Weights are split into
    per-chunk tiles and DMA-ordered so the PE starts as soon as the
    first chunks land. Also combines the pass-1 aux partials into the
    scalar aux loss on-device.
  Host combine: scatter-add of the (already router-weighted) per-expert
    outputs back to [B, T, D]. Each token receives exactly its two
    expert contributions.
"""

import os
import numpy as np

import concourse.bass as bass
import concourse.mybir as mybir
import concourse.tile as tile
from concourse import bacc
from concourse.alu_op_type import AluOpType
from concourse.bass_utils import run_bass_kernel_spmd
from concourse.masks import make_identity

f32 = mybir.dt.float32
f32r = mybir.dt.float32r
AX = mybir.AxisListType
ACT = mybir.ActivationFunctionType
TT = AluOpType

B, T, D, E, F = 2, 2048, 1024, 8, 2048
N = B * T           # 4096 tokens
NCORES = 8
TPC = N // NCORES   # 512 tokens per core in pass 1
DCH = D // 128      # 8 d-chunks
FCH = F // 128      # 16 f-chunks
TBLK = 384          # pass-2 token block (moving dim; >=256 keeps f32r fast)

Z_LOSS_COEF = 1e-3

_nc_cache = {}


def _build_pass1():
    """Router kernel: one core's 512-token shard."""
    nc = bacc.Bacc("TRN2", target_bir_lowering=False, debug=False,
                   num_devices=NCORES)
    xt_d = nc.dram_tensor("xt", [D, TPC], f32r, kind="ExternalInput").ap()
    gwt_d = nc.dram_tensor("gwt", [128, DCH * E], f32r, kind="ExternalInput").ap()
    wfull_d = nc.dram_tensor("wfull", [TPC, E], f32, kind="ExternalOutput").ap()
    auxp_d = nc.dram_tensor("auxp", [1, 17], f32, kind="ExternalOutput").ap()

    NB = TPC // 128  # 4 token blocks per core

    with tile.TileContext(nc) as tc:
        with tc.tile_pool(name="sb", bufs=1) as sb, \
             tc.tile_pool(name="ps", bufs=2, space="PSUM") as ps, \
             tc.tile_pool(name="pl", bufs=1, space="PSUM") as pl:
            gw = sb.tile([128, DCH, E], f32r)
            nc.gpsimd.dma_start(out=gw[:], in_=gwt_d.rearrange("p (c e) -> p c e", e=E))
            xts = []
            qs = [nc.sync, nc.scalar]
            for d in range(DCH):
                xtd = sb.tile([128, TPC], f32r, name=f"xt{d}", tag=f"xt{d}")
                qs[d % 2].dma_start(out=xtd[:], in_=xt_d[bass.ts(d, 128), :])
                xts.append(xtd)
            ident = sb.tile([128, 128], f32)
            make_identity(nc, ident[:])
            ones = sb.tile([128, 1], f32)
            nc.vector.memset(ones[:], 1.0)
            # PE warm-up during the DMA window (HAM needs ~3.4us of
            # activity to unthrottle 1.2 -> 2.4 GHz)
            wsc = sb.tile([128, 128], f32)
            nc.vector.memset(wsc[:], 1.0)
            wpsA = ps.tile([128, 128], f32, tag="lgp", name="wpsA")
            wpsB = ps.tile([128, 128], f32, tag="lgp", name="wpsB")
            for i in range(10):
                nc.tensor.matmul((wpsA if i % 2 == 0 else wpsB)[:],
                                 lhsT=wsc[:], rhs=wsc[:], start=True, stop=True)

            # logits^T [E, TPC] on PSUM (gate stationary, tokens moving, fp32)
            lgT = pl.tile([E, TPC], f32, tag="lgT")
            for d in range(DCH):
                nc.tensor.matmul(lgT[:], lhsT=gw[:, d, :], rhs=xts[d][:],
                                 start=(d == 0), stop=(d == DCH - 1))
            lgTs = sb.tile([E, TPC], f32)
            nc.scalar.copy(lgTs[:], lgT[:])
            # transpose back to [128 tokens, E] per block, gather into SBUF
            lgs = sb.tile([128, NB, E], f32)
            for b in range(NB):
                lgp = ps.tile([128, E], f32, tag="lgp")
                nc.tensor.transpose(lgp[:], lgTs[:, bass.ts(b, 128)],
                                    ident[:E, :E])
                nc.scalar.copy(lgs[:, b, :], lgp[:])

            def bc(ap):  # [128, NB] -> [128, NB, E] stride-0 broadcast
                return ap[:, :, None].broadcast_to([128, NB, E])

            acc = sb.tile([128, 17], f32)   # [probs 8 | onehot 8 | z 1]
            t1 = sb.tile([128, NB], f32)
            nc.vector.tensor_reduce(t1[:], lgs[:], axis=AX.X, op=TT.max)
            eq1 = sb.tile([128, NB, E], f32)
            nc.vector.tensor_tensor(eq1[:], lgs[:], bc(t1), TT.is_equal)
            msk = sb.tile([128, NB, E], f32)
            nc.vector.scalar_tensor_tensor(out=msk[:], in0=eq1[:], scalar=-1e30,
                                           in1=lgs[:], op0=TT.mult, op1=TT.add)
            t2 = sb.tile([128, NB], f32)
            nc.vector.tensor_reduce(t2[:], msk[:], axis=AX.X, op=TT.max)
            eq2 = sb.tile([128, NB, E], f32)
            nc.vector.tensor_tensor(eq2[:], msk[:], bc(t2), TT.is_equal)
            # top-2 softmax weights: wa = 1/(1+exp(t2-t1)), wb = 1-wa
            d21 = sb.tile([128, NB], f32)
            nc.vector.tensor_sub(d21[:], t2[:], t1[:])
            ex = sb.tile([128, NB], f32)
            nc.scalar.activation(ex[:], d21[:], ACT.Exp)
            den = sb.tile([128, NB], f32)
            nc.vector.tensor_scalar_add(den[:], ex[:], 1.0)
            wa = sb.tile([128, NB], f32)
            nc.vector.reciprocal(wa[:], den[:])
            wb = sb.tile([128, NB], f32)
            nc.vector.tensor_mul(wb[:], ex[:], wa[:])
            # wfull = wa*eq1 + wb*eq2
            wf = sb.tile([128, NB, E], f32)
            nc.vector.tensor_tensor(wf[:], eq1[:], bc(wa), TT.mult)
            wf2 = sb.tile([128, NB, E], f32)
            nc.vector.tensor_tensor(wf2[:], eq2[:], bc(wb), TT.mult)
            nc.vector.tensor_add(wf[:], wf[:], wf2[:])
            nc.sync.dma_start(out=wfull_d.rearrange("(b p) e -> p b e", p=128),
                              in_=wf[:])
            # softmax probs for aux
            sub = sb.tile([128, NB, E], f32)
            nc.vector.tensor_tensor(sub[:], lgs[:], bc(t1), TT.subtract)
            pr = sb.tile([128, NB, E], f32)
            nc.scalar.activation(pr[:], sub[:], ACT.Exp)
            se = sb.tile([128, NB], f32)
            nc.vector.tensor_reduce(se[:], pr[:], axis=AX.X, op=TT.add)
            rs = sb.tile([128, NB], f32)
            nc.vector.reciprocal(rs[:], se[:])
            prn = sb.tile([128, NB, E], f32)
            nc.vector.tensor_tensor(prn[:], pr[:], bc(rs), TT.mult)
            # block-sums into acc
            nc.vector.tensor_add(acc[:, 0:E], prn[:, 0, :], prn[:, 1, :])
            nc.vector.tensor_add(acc[:, 0:E], acc[:, 0:E], prn[:, 2, :])
            nc.vector.tensor_add(acc[:, 0:E], acc[:, 0:E], prn[:, 3, :])
            nc.vector.tensor_add(acc[:, E:2 * E], eq1[:, 0, :], eq1[:, 1, :])
            nc.vector.tensor_add(acc[:, E:2 * E], acc[:, E:2 * E], eq1[:, 2, :])
            nc.vector.tensor_add(acc[:, E:2 * E], acc[:, E:2 * E], eq1[:, 3, :])
            sq = sb.tile([128, NB, E], f32)
            nc.scalar.activation(sq[:], lgs[:], ACT.Square,
                                 accum_out=acc[:, 16:17])
            # column sums over the 128 partitions via ones-matmul
            accp = ps.tile([1, 17], f32, tag="accp")
            nc.tensor.matmul(accp[:], lhsT=ones[:], rhs=acc[:], start=True,
                             stop=True)
            accs = sb.tile([1, 17], f32)
            nc.vector.tensor_copy(accs[:], accp[:])
            nc.sync.dma_start(out=auxp_d[:], in_=accs[:])

    nc.compile()
    return nc


def _build_pass2(C, has_b2=True):
    """Expert FFN kernel: one expert's C gathered tokens (C % TBLK == 0)."""
    nc = bacc.Bacc("TRN2", target_bir_lowering=False, debug=False,
                   num_devices=NCORES)
    xgt_d = nc.dram_tensor("xgt", [D, C], f32r, kind="ExternalInput").ap()
    w1_d = nc.dram_tensor("w1", [D, F], f32r, kind="ExternalInput").ap()
    w2_d = nc.dram_tensor("w2", [F, D], f32r, kind="ExternalInput").ap()
    b1t_d = nc.dram_tensor("b1t", [128, FCH], f32, kind="ExternalInput").ap()
    b2b_d = nc.dram_tensor("b2b", [128, D], f32, kind="ExternalInput").ap()
    wtok_d = nc.dram_tensor("wtok", [128, C // 128], f32, kind="ExternalInput").ap()
    auxp_d = nc.dram_tensor("auxp", [NCORES, 17], f32, kind="ExternalInput").ap()
    y_d = nc.dram_tensor("y", [C, D], f32, kind="ExternalOutput").ap()
    aux_d = nc.dram_tensor("aux", [1, 1], f32, kind="ExternalOutput").ap()

    nblk = C // TBLK
    nsub = TBLK // 128

    with tile.TileContext(nc) as tc:
        with tc.tile_pool(name="wt", bufs=1) as wt, \
             tc.tile_pool(name="xg", bufs=2) as xg, \
             tc.tile_pool(name="hp", bufs=1) as hp, \
             tc.tile_pool(name="yo", bufs=2) as yo, \
             tc.tile_pool(name="sm", bufs=1) as sm, \
             tc.tile_pool(name="ph", bufs=6, space="PSUM") as ph, \
             tc.tile_pool(name="py", bufs=2, space="PSUM") as py:

            def xgt_block(t):
                xts = []
                for d in range(DCH):
                    xtd = xg.tile([128, TBLK], f32r, name=f"xt{t}_{d}",
                                  tag=f"xtd{d}")
                    nc.sync.dma_start(
                        out=xtd[:],
                        in_=xgt_d[bass.ts(d, 128), bass.ts(t, TBLK)])
                    xts.append(xtd)
                return xts

            # DMA issue order: first block's tokens + w1 first (mm1 needs
            # them), then small constants, then w2 (mm2 starts ~25us in).
            xts0 = xgt_block(0)
            w1c = []
            for d in range(DCH):
                w1d = wt.tile([128, F], f32r, name=f"w1_{d}")
                nc.sync.dma_start(out=w1d[:], in_=w1_d[bass.ts(d, 128), :])
                w1c.append(w1d)
            b1t = wt.tile([128, FCH], f32)
            nc.sync.dma_start(out=b1t[:], in_=b1t_d[:])
            wtok = wt.tile([128, C // 128], f32)
            nc.sync.dma_start(out=wtok[:], in_=wtok_d[:])
            w2c = []
            for f in range(FCH):
                w2f = wt.tile([128, D], f32r, name=f"w2_{f}")
                nc.sync.dma_start(out=w2f[:], in_=w2_d[bass.ts(f, 128), :])
                w2c.append(w2f)
            if has_b2:
                b2b = wt.tile([128, D], f32)
                nc.gpsimd.dma_start(out=b2b[:], in_=b2b_d[:])

            # PE warm-up during the weight-stream window
            wsc = sm.tile([128, 128], f32)
            nc.vector.memset(wsc[:], 1.0)
            wpsA = py.tile([128, 128], f32, tag="yps", name="wpsA")
            wpsB = py.tile([128, 128], f32, tag="yps", name="wpsB")
            for i in range(14):
                nc.tensor.matmul((wpsA if i % 2 == 0 else wpsB)[:],
                                 lhsT=wsc[:], rhs=wsc[:], start=True, stop=True)

            # aux combine (tiny, once)
            auxp = sm.tile([NCORES, 17], f32)
            nc.gpsimd.dma_start(out=auxp[:], in_=auxp_d[:])
            ones8 = sm.tile([NCORES, 1], f32)
            nc.vector.memset(ones8[:], 1.0)
            auxs = sm.tile([1, 17], f32)
            auxt = py.tile([1, 17], f32, tag="yps", name="auxt")
            nc.tensor.matmul(auxt[:], lhsT=ones8[:], rhs=auxp[:], start=True,
                             stop=True)
            nc.vector.tensor_copy(auxs[:], auxt[:])
            prod = sm.tile([1, E], f32)
            nc.vector.tensor_mul(prod[:], auxs[:, 0:E], auxs[:, E:2 * E])
            psum_ = sm.tile([1, 1], f32)
            nc.vector.tensor_reduce(psum_[:], prod[:], axis=AX.X, op=TT.add)
            zt = sm.tile([1, 1], f32)
            nc.vector.tensor_scalar_mul(zt[:], auxs[:, 16:17],
                                        float(Z_LOSS_COEF / (N * E)))
            auxo = sm.tile([1, 1], f32)
            nc.vector.scalar_tensor_tensor(out=auxo[:], in0=psum_[:],
                                           scalar=float(E) / (float(N) * float(N)),
                                           in1=zt[:], op0=TT.mult, op1=TT.add)
            nc.gpsimd.dma_start(out=aux_d[:], in_=auxo[:])

            # main FFN loop. mm1 runs d-outer over f-groups of <=6 so the
            # PE consumes w1/xgt chunks in DMA-arrival order on block 0.
            FG = [list(range(0, 6)), list(range(6, 12)), list(range(12, 16))]
            for t in range(nblk):
                xts = xts0 if t == 0 else xgt_block(t)
                hT = hp.tile([128, FCH, TBLK], f32r, tag="hT")
                for fg in FG:
                    hps_l = [ph.tile([128, TBLK], f32, tag="hps",
                                     name=f"hps{t}_{f}") for f in fg]
                    for d in range(DCH):
                        for j, f in enumerate(fg):
                            nc.tensor.matmul(hps_l[j][:],
                                             lhsT=w1c[d][:, bass.ts(f, 128)],
                                             rhs=xts[d][:],
                                             start=(d == 0), stop=(d == DCH - 1))
                    # hT = gelu(w1.T x + b1), erf flavor
                    for j, f in enumerate(fg):
                        nc.scalar.activation(hT[:, f, :], hps_l[j][:], ACT.Gelu,
                                             bias=b1t[:, f:f + 1])
                for s in range(nsub):
                    yt = yo.tile([128, D], f32, tag="yt")
                    for n in range(2):
                        yps = py.tile([128, 512], f32, tag="yps")
                        for f in range(FCH):
                            nc.tensor.matmul(yps[:],
                                             lhsT=hT[:, f, bass.ts(s, 128)],
                                             rhs=w2c[f][:, bass.ts(n, 512)],
                                             start=(f == 0), stop=(f == FCH - 1))
                        # y = wtok * (psum + b2)
                        if has_b2:
                            tb = yo.tile([128, 512], f32, tag="tb")
                            nc.vector.tensor_add(tb[:], yps[:],
                                                 b2b[:, bass.ts(n, 512)])
                            src_ap = tb[:]
                        else:
                            src_ap = yps[:]
                        nc.scalar.activation(
                            yt[:, bass.ts(n, 512)], src_ap, ACT.Copy,
                            scale=wtok[:, t * nsub + s:t * nsub + s + 1])
                        nc.scalar.dma_start(
                            out=y_d[bass.ts(t * nsub + s, 128), bass.ts(n, 512)],
                            in_=yt[:, bass.ts(n, 512)])

    nc.compile()
    return nc


def _get_pass1():
    if "p1" not in _nc_cache:
        _nc_cache["p1"] = _build_pass1()
    return _nc_cache["p1"]


def _get_pass2(C, has_b2=True):
    key = ("p2", C, has_b2)
    if key not in _nc_cache:
        _nc_cache[key] = _build_pass2(C, has_b2)
    return _nc_cache[key]


def run(inputs, trace=False, trace_cores=None):
    x = np.ascontiguousarray(np.asarray(inputs["x"], dtype=np.float32))
    gate_w = np.ascontiguousarray(np.asarray(inputs["gate_w"], dtype=np.float32))
    ew1 = np.ascontiguousarray(np.asarray(inputs["expert_w1"], dtype=np.float32))
    eb1 = np.ascontiguousarray(np.asarray(inputs["expert_b1"], dtype=np.float32))
    ew2 = np.ascontiguousarray(np.asarray(inputs["expert_w2"], dtype=np.float32))
    eb2 = np.ascontiguousarray(np.asarray(inputs["expert_b2"], dtype=np.float32))

    xf = x.reshape(N, D)
    xT = np.ascontiguousarray(xf.T)                       # [D, N]
    gwt = np.ascontiguousarray(
        gate_w.reshape(DCH, 128, E).transpose(1, 0, 2).reshape(128, DCH * E))

    perf = {}
    kw = dict(trace=trace)
    if trace and trace_cores is not None:
        kw["trace_cores"] = trace_cores

    # ---- pass 1: router ----
    nc1 = _get_pass1()
    in1 = [{"xt": np.ascontiguousarray(xT[:, c * TPC:(c + 1) * TPC]), "gwt": gwt}
           for c in range(NCORES)]
    r1 = run_bass_kernel_spmd(nc1, in1, core_ids=list(range(NCORES)), **kw)
    perf["pass1_ns"] = r1.exec_time_ns
    wfull = np.concatenate([r1.results[c]["wfull"] for c in range(NCORES)], axis=0)
    auxp = np.concatenate([r1.results[c]["auxp"] for c in range(NCORES)], axis=0)

    # ---- host dispatch (data movement only) ----
    idx = [np.nonzero(wfull[:, e])[0] for e in range(E)]
    maxc = max(len(i) for i in idx)
    C = max(TBLK, ((maxc + TBLK - 1) // TBLK) * TBLK)
    in2 = []
    for e in range(E):
        ie = idx[e]
        xg = np.zeros((D, C), np.float32)
        xg[:, :len(ie)] = xT[:, ie]
        wt = np.zeros(C, np.float32)
        wt[:len(ie)] = wfull[ie, e]
        in2.append({
            "xgt": xg,
            "w1": ew1[e],
            "w2": ew2[e],
            "b1t": np.ascontiguousarray(eb1[e].reshape(FCH, 128).T),
            "b2b": np.ascontiguousarray(np.broadcast_to(eb2[e], (128, D))),
            "wtok": np.ascontiguousarray(wt.reshape(C // 128, 128).T),
            "auxp": auxp,
        })

    # ---- pass 2: expert FFN ----
    has_b2 = bool(np.any(eb2))
    nc2 = _get_pass2(C, has_b2)
    r2 = run_bass_kernel_spmd(nc2, in2, core_ids=list(range(NCORES)), **kw)
    perf["pass2_ns"] = r2.exec_time_ns
    perf["C"] = C
    perf["r1"] = r1
    perf["r2"] = r2

    # ---- host combine (scatter-add of the two expert contributions) ----
    out = np.zeros((N, D), np.float32)
    for e in range(E):
        ie = idx[e]
        out[ie] += r2.results[e]["y"][:len(ie)]
    aux = np.float32(r2.results[0]["aux"][0, 0])
    return out.reshape(B, T, D), aux, perf


def kernel(**inputs):
    out, aux, _ = run(inputs, trace=bool(int(os.environ.get("KERNEL_TRACE", "0"))))
    return out, aux


# revision 16
# speedup vs baseline: 1.0672x; 1.0370x over previous
"""MoE FFN (top-2 routing) Trainium2 kernel.

Strategy (8 NeuronCores, SPMD via run_bass_kernel_spmd):
  Pass 1 (router, data-parallel over tokens): each core takes N/8 = 512
    tokens (x pre-transposed to [D, 512] by the host), computes logits =
    x @ gate_w in fp32 on the PE (gate stationary, tokens moving, then a
    PE transpose back to token-partition layout), derives the top-2
    expert mask and softmax weights on-device with batched DVE ops, and
    emits:
      - wfull [512, E]: per-token router weight for every expert
        (nonzero exactly at the token's top-2 experts)
      - aux partials [1, 17]: softmax-prob column sums (8), top-1
        one-hot column sums (8), sum of logits^2 (1)
  Host dispatch ("all-to-all"): tokens are gathered per expert from the
    nonzero pattern of wfull, laid out transposed ([D, C], zero-padded
    to capacity C), and shipped to the expert's core. Pure data
    movement - no arithmetic on the host.
  Pass 2 (expert FFN, expert-parallel): core e holds expert e's w1/w2
    resident in SBUF and streams its gathered tokens through
      hT = gelu(w1.T @ xT + b1)   [F on partitions]
      y  = wtok * (hT.T @ w2 + b2) [tokens on partitions]
    with float32r matmuls (fp32 data rounded to 11-bit mantissa, 1
    cycle/row on the PE for free dim >= 256). Weights are split into
    per-chunk tiles and DMA-ordered so the PE starts as soon as the
    first chunks land. Also combines the pass-1 aux partials into the
    scalar aux loss on-device.
  Host combine: scatter-add of the (already router-weighted) per-expert
    outputs back to [B, T, D]. Each token receives exactly its two
    expert contributions.
"""

import os
import numpy as np

import concourse.bass as bass
import concourse.mybir as mybir
import concourse.tile as tile
from concourse import bacc
from concourse.alu_op_type import AluOpType
from concourse.bass_utils import run_bass_kernel_spmd
from concourse.masks import make_identity

f32 = mybir.dt.float32
f32r = mybir.dt.float32r
AX = mybir.AxisListType
ACT = mybir.ActivationFunctionType
TT = AluOpType

B, T, D, E, F = 2, 2048, 1024, 8, 2048
N = B * T           # 4096 tokens
NCORES = 8
TPC = N // NCORES   # 512 tokens per core in pass 1
DCH = D // 128      # 8 d-chunks
FCH = F // 128      # 16 f-chunks
TBLK = 384          # pass-2 token block (moving dim; >=256 keeps f32r fast)

Z_LOSS_COEF = 1e-3

_nc_cache = {}


def _build_pass1():
    """Router kernel: one core's 512-token shard."""
    nc = bacc.Bacc("TRN2", target_bir_lowering=False, debug=False,
                   num_devices=NCORES)
    xt_d = nc.dram_tensor("xt", [D, TPC], f32r, kind="ExternalInput").ap()
    gwt_d = nc.dram_tensor("gwt", [128, DCH * E], f32r, kind="ExternalInput").ap()
    wfull_d = nc.dram_tensor("wfull", [TPC, E], f32, kind="ExternalOutput").ap()
    auxp_d = nc.dram_tensor("auxp", [1, 17], f32, kind="ExternalOutput").ap()

    NB = TPC // 128  # 4 token blocks per core

    with tile.TileContext(nc) as tc:
        with tc.tile_pool(name="sb", bufs=1) as sb, \
             tc.tile_pool(name="ps", bufs=2, space="PSUM") as ps, \
             tc.tile_pool(name="pl", bufs=1, space="PSUM") as pl:
            gw = sb.tile([128, DCH, E], f32r)
            nc.gpsimd.dma_start(out=gw[:], in_=gwt_d.rearrange("p (c e) -> p c e", e=E))
            xts = []
            qs = [nc.sync, nc.scalar]
            for d in range(DCH):
                xtd = sb.tile([128, TPC], f32r, name=f"xt{d}", tag=f"xt{d}")
                qs[d % 2].dma_start(out=xtd[:], in_=xt_d[bass.ts(d, 128), :])
                xts.append(xtd)
            ident = sb.tile([128, 128], f32)
            make_identity(nc, ident[:])
            ones = sb.tile([128, 1], f32)
            nc.vector.memset(ones[:], 1.0)
            # PE warm-up during the DMA window (HAM needs ~3.4us of
            # activity to unthrottle 1.2 -> 2.4 GHz)
            wsc = sb.tile([128, 128], f32)
            nc.vector.memset(wsc[:], 1.0)
            wpsA = ps.tile([128, 128], f32, tag="lgp", name="wpsA")
            wpsB = ps.tile([128, 128], f32, tag="lgp", name="wpsB")
            for i in range(10):
                nc.tensor.matmul((wpsA if i % 2 == 0 else wpsB)[:],
                                 lhsT=wsc[:], rhs=wsc[:], start=True, stop=True)

            # logits^T [E, TPC] on PSUM (gate stationary, tokens moving, fp32)
            lgT = pl.tile([E, TPC], f32, tag="lgT")
            for d in range(DCH):
                nc.tensor.matmul(lgT[:], lhsT=gw[:, d, :], rhs=xts[d][:],
                                 start=(d == 0), stop=(d == DCH - 1))
            lgTs = sb.tile([E, TPC], f32)
            nc.scalar.copy(lgTs[:], lgT[:])
            # transpose back to [128 tokens, E] per block, gather into SBUF
            lgs = sb.tile([128, NB, E], f32)
            for b in range(NB):
                lgp = ps.tile([128, E], f32, tag="lgp")
                nc.tensor.transpose(lgp[:], lgTs[:, bass.ts(b, 128)],
                                    ident[:E, :E])
                nc.scalar.copy(lgs[:, b, :], lgp[:])

            def bc(ap):  # [128, NB] -> [128, NB, E] stride-0 broadcast
                return ap[:, :, None].broadcast_to([128, NB, E])

            acc = sb.tile([128, 17], f32)   # [probs 8 | onehot 8 | z 1]
            t1 = sb.tile([128, NB], f32)
            nc.vector.tensor_reduce(t1[:], lgs[:], axis=AX.X, op=TT.max)
            eq1 = sb.tile([128, NB, E], f32)
            nc.vector.tensor_tensor(eq1[:], lgs[:], bc(t1), TT.is_equal)
            msk = sb.tile([128, NB, E], f32)
            nc.vector.scalar_tensor_tensor(out=msk[:], in0=eq1[:], scalar=-1e30,
                                           in1=lgs[:], op0=TT.mult, op1=TT.add)
            t2 = sb.tile([128, NB], f32)
            nc.vector.tensor_reduce(t2[:], msk[:], axis=AX.X, op=TT.max)
            eq2 = sb.tile([128, NB, E], f32)
            nc.vector.tensor_tensor(eq2[:], msk[:], bc(t2), TT.is_equal)
            # top-2 softmax weights: wa = 1/(1+exp(t2-t1)), wb = 1-wa
            d21 = sb.tile([128, NB], f32)
            nc.vector.tensor_sub(d21[:], t2[:], t1[:])
            ex = sb.tile([128, NB], f32)
            nc.scalar.activation(ex[:], d21[:], ACT.Exp)
            den = sb.tile([128, NB], f32)
            nc.vector.tensor_scalar_add(den[:], ex[:], 1.0)
            wa = sb.tile([128, NB], f32)
            nc.vector.reciprocal(wa[:], den[:])
            wb = sb.tile([128, NB], f32)
            nc.vector.tensor_mul(wb[:], ex[:], wa[:])
            # wfull = wa*eq1 + wb*eq2
            wf = sb.tile([128, NB, E], f32)
            nc.vector.tensor_tensor(wf[:], eq1[:], bc(wa), TT.mult)
            wf2 = sb.tile([128, NB, E], f32)
            nc.vector.tensor_tensor(wf2[:], eq2[:], bc(wb), TT.mult)
            nc.vector.tensor_add(wf[:], wf[:], wf2[:])
            nc.sync.dma_start(out=wfull_d.rearrange("(b p) e -> p b e", p=128),
                              in_=wf[:])
            # softmax probs for aux
            sub = sb.tile([128, NB, E], f32)
            nc.vector.tensor_tensor(sub[:], lgs[:], bc(t1), TT.subtract)
            pr = sb.tile([128, NB, E], f32)
            nc.scalar.activation(pr[:], sub[:], ACT.Exp)
            se = sb.tile([128, NB], f32)
            nc.vector.tensor_reduce(se[:], pr[:], axis=AX.X, op=TT.add)
            rs = sb.tile([128, NB], f32)
            nc.vector.reciprocal(rs[:], se[:])
            prn = sb.tile([128, NB, E], f32)
            nc.vector.tensor_tensor(prn[:], pr[:], bc(rs), TT.mult)
            # block-sums into acc
            nc.vector.tensor_add(acc[:, 0:E], prn[:, 0, :], prn[:, 1, :])
            nc.vector.tensor_add(acc[:, 0:E], acc[:, 0:E], prn[:, 2, :])
            nc.vector.tensor_add(acc[:, 0:E], acc[:, 0:E], prn[:, 3, :])
            nc.vector.tensor_add(acc[:, E:2 * E], eq1[:, 0, :], eq1[:, 1, :])
            nc.vector.tensor_add(acc[:, E:2 * E], acc[:, E:2 * E], eq1[:, 2, :])
            nc.vector.tensor_add(acc[:, E:2 * E], acc[:, E:2 * E], eq1[:, 3, :])
            sq = sb.tile([128, NB, E], f32)
            nc.scalar.activation(sq[:], lgs[:], ACT.Square,
                                 accum_out=acc[:, 16:17])
            # column sums over the 128 partitions via ones-matmul
            accp = ps.tile([1, 17], f32, tag="accp")
            nc.tensor.matmul(accp[:], lhsT=ones[:], rhs=acc[:], start=True,
                             stop=True)
            accs = sb.tile([1, 17], f32)
            nc.vector.tensor_copy(accs[:], accp[:])
            nc.sync.dma_start(out=auxp_d[:], in_=accs[:])

    nc.compile()
    return nc


def _build_pass2(C, has_b2=True):
    """Expert FFN kernel: one expert's C gathered tokens (C % TBLK == 0)."""
    nc = bacc.Bacc("TRN2", target_bir_lowering=False, debug=False,
                   num_devices=NCORES)
    xgt_d = nc.dram_tensor("xgt", [D, C], f32r, kind="ExternalInput").ap()
    w1_d = nc.dram_tensor("w1", [D, F], f32r, kind="ExternalInput").ap()
    w2_d = nc.dram_tensor("w2", [F, D], f32r, kind="ExternalInput").ap()
    b1t_d = nc.dram_tensor("b1t", [128, FCH], f32, kind="ExternalInput").ap()
    b2b_d = nc.dram_tensor("b2b", [128, D], f32, kind="ExternalInput").ap()
    wtok_d = nc.dram_tensor("wtok", [128, C // 128], f32, kind="ExternalInput").ap()
    auxp_d = nc.dram_tensor("auxp", [NCORES, 17], f32, kind="ExternalInput").ap()
    y_d = nc.dram_tensor("y", [C, D], f32, kind="ExternalOutput").ap()
    aux_d = nc.dram_tensor("aux", [1, 1], f32, kind="ExternalOutput").ap()

    nblk = C // TBLK
    nsub = TBLK // 128

    with tile.TileContext(nc) as tc:
        with tc.tile_pool(name="wt", bufs=1) as wt, \
             tc.tile_pool(name="xg", bufs=2) as xg, \
             tc.tile_pool(name="hp", bufs=1) as hp, \
             tc.tile_pool(name="yo", bufs=2) as yo, \
             tc.tile_pool(name="sm", bufs=1) as sm, \
             tc.tile_pool(name="ph", bufs=6, space="PSUM") as ph, \
             tc.tile_pool(name="py", bufs=2, space="PSUM") as py:

            def xgt_block(t):
                xts = []
                for d in range(DCH):
                    xtd = xg.tile([128, TBLK], f32r, name=f"xt{t}_{d}",
                                  tag=f"xtd{d}")
                    nc.sync.dma_start(
                        out=xtd[:],
                        in_=xgt_d[bass.ts(d, 128), bass.ts(t, TBLK)])
                    xts.append(xtd)
                return xts

            # DMA issue order: first block's tokens + w1 first (mm1 needs
            # them), then small constants, then w2 (mm2 starts ~25us in).
            xts0 = xgt_block(0)
            w1c = []
            for d in range(DCH):
                w1d = wt.tile([128, F], f32r, name=f"w1_{d}")
                nc.sync.dma_start(out=w1d[:], in_=w1_d[bass.ts(d, 128), :])
                w1c.append(w1d)
            b1t = wt.tile([128, FCH], f32)
            nc.sync.dma_start(out=b1t[:], in_=b1t_d[:])
            wtok = wt.tile([128, C // 128], f32)
            nc.sync.dma_start(out=wtok[:], in_=wtok_d[:])
            w2c = []
            for f in range(FCH):
                w2f = wt.tile([128, D], f32r, name=f"w2_{f}")
                nc.sync.dma_start(out=w2f[:], in_=w2_d[bass.ts(f, 128), :])
                w2c.append(w2f)
            if has_b2:
                b2b = wt.tile([128, D], f32)
                nc.gpsimd.dma_start(out=b2b[:], in_=b2b_d[:])

            # PE warm-up during the weight-stream window
            wsc = sm.tile([128, 128], f32)
            nc.vector.memset(wsc[:], 1.0)
            wpsA = py.tile([128, 128], f32, tag="yps", name="wpsA")
            wpsB = py.tile([128, 128], f32, tag="yps", name="wpsB")
            for i in range(14):
                nc.tensor.matmul((wpsA if i % 2 == 0 else wpsB)[:],
                                 lhsT=wsc[:], rhs=wsc[:], start=True, stop=True)

            # aux combine (tiny, once)
            auxp = sm.tile([NCORES, 17], f32)
            nc.gpsimd.dma_start(out=auxp[:], in_=auxp_d[:])
            ones8 = sm.tile([NCORES, 1], f32)
            nc.vector.memset(ones8[:], 1.0)
            auxs = sm.tile([1, 17], f32)
            auxt = py.tile([1, 17], f32, tag="yps", name="auxt")
            nc.tensor.matmul(auxt[:], lhsT=ones8[:], rhs=auxp[:], start=True,
                             stop=True)
            nc.vector.tensor_copy(auxs[:], auxt[:])
            prod = sm.tile([1, E], f32)
            nc.vector.tensor_mul(prod[:], auxs[:, 0:E], auxs[:, E:2 * E])
            psum_ = sm.tile([1, 1], f32)
            nc.vector.tensor_reduce(psum_[:], prod[:], axis=AX.X, op=TT.add)
            zt = sm.tile([1, 1], f32)
            nc.vector.tensor_scalar_mul(zt[:], auxs[:, 16:17],
                                        float(Z_LOSS_COEF / (N * E)))
            auxo = sm.tile([1, 1], f32)
            nc.vector.scalar_tensor_tensor(out=auxo[:], in0=psum_[:],
                                           scalar=float(E) / (float(N) * float(N)),
                                           in1=zt[:], op0=TT.mult, op1=TT.add)
            nc.gpsimd.dma_start(out=aux_d[:], in_=auxo[:])

            # main FFN loop. mm1 runs d-outer over f-groups of <=6 so the
            # PE consumes w1/xgt chunks in DMA-arrival order on block 0.
            # Block 0 additionally splits the d-contraction in half
            # (partials parked in SBUF) so psum groups complete after only
            # half the w1 stream and the PE stays busy while w1 lands.
            FG = [list(range(0, 6)), list(range(6, 12)), list(range(12, 16))]
            for t in range(nblk):
                xts = xts0 if t == 0 else xgt_block(t)
                hT = hp.tile([128, FCH, TBLK], f32r, tag="hT")
                if t == 0:
                    # half-d partials parked in hT itself (f32r rounding of
                    # a partial sum costs ~6e-5 relative, negligible)
                    for fg in FG:
                        hps_l = [ph.tile([128, TBLK], f32, tag="hps",
                                         name=f"hpsA_{f}") for f in fg]
                        for d in range(DCH // 2):
                            for j, f in enumerate(fg):
                                nc.tensor.matmul(hps_l[j][:],
                                                 lhsT=w1c[d][:, bass.ts(f, 128)],
                                                 rhs=xts[d][:],
                                                 start=(d == 0),
                                                 stop=(d == DCH // 2 - 1))
                        for j, f in enumerate(fg):
                            nc.scalar.copy(hT[:, f, :], hps_l[j][:])
                    for fg in FG:
                        hps_l = [ph.tile([128, TBLK], f32, tag="hps",
                                         name=f"hpsB_{f}") for f in fg]
                        for d in range(DCH // 2, DCH):
                            for j, f in enumerate(fg):
                                nc.tensor.matmul(hps_l[j][:],
                                                 lhsT=w1c[d][:, bass.ts(f, 128)],
                                                 rhs=xts[d][:],
                                                 start=(d == DCH // 2),
                                                 stop=(d == DCH - 1))
                        for j, f in enumerate(fg):
                            nc.vector.tensor_add(hT[:, f, :], hps_l[j][:],
                                                 hT[:, f, :])
                            nc.scalar.activation(hT[:, f, :], hT[:, f, :],
                                                 ACT.Gelu, bias=b1t[:, f:f + 1])
                else:
                    for fg in FG:
                        hps_l = [ph.tile([128, TBLK], f32, tag="hps",
                                         name=f"hps{t}_{f}") for f in fg]
                        for d in range(DCH):
                            for j, f in enumerate(fg):
                                nc.tensor.matmul(hps_l[j][:],
                                                 lhsT=w1c[d][:, bass.ts(f, 128)],
                                                 rhs=xts[d][:],
                                                 start=(d == 0),
                                                 stop=(d == DCH - 1))
                        # hT = gelu(w1.T x + b1), erf flavor
                        for j, f in enumerate(fg):
                            nc.scalar.activation(hT[:, f, :], hps_l[j][:],
                                                 ACT.Gelu, bias=b1t[:, f:f + 1])
                for s in range(nsub):
                    yt = yo.tile([128, D], f32, tag="yt")
                    for n in range(2):
                        yps = py.tile([128, 512], f32, tag="yps")
                        for f in range(FCH):
                            nc.tensor.matmul(yps[:],
                                             lhsT=hT[:, f, bass.ts(s, 128)],
                                             rhs=w2c[f][:, bass.ts(n, 512)],
                                             start=(f == 0), stop=(f == FCH - 1))
                        # y = wtok * (psum + b2)
                        if has_b2:
                            tb = yo.tile([128, 512], f32, tag="tb")
                            nc.vector.tensor_add(tb[:], yps[:],
                                                 b2b[:, bass.ts(n, 512)])
                            src_ap = tb[:]
                        else:
                            src_ap = yps[:]
                        nc.scalar.activation(
                            yt[:, bass.ts(n, 512)], src_ap, ACT.Copy,
                            scale=wtok[:, t * nsub + s:t * nsub + s + 1])
                        nc.scalar.dma_start(
                            out=y_d[bass.ts(t * nsub + s, 128), bass.ts(n, 512)],
                            in_=yt[:, bass.ts(n, 512)])

    nc.compile()
    return nc


def _get_pass1():
    if "p1" not in _nc_cache:
        _nc_cache["p1"] = _build_pass1()
    return _nc_cache["p1"]


def _get_pass2(C, has_b2=True):
    key = ("p2", C, has_b2)
    if key not in _nc_cache:
        _nc_cache[key] = _build_pass2(C, has_b2)
    return _nc_cache[key]


def run(inputs, trace=False, trace_cores=None):
    x = np.ascontiguousarray(np.asarray(inputs["x"], dtype=np.float32))
    gate_w = np.ascontiguousarray(np.asarray(inputs["gate_w"], dtype=np.float32))
    ew1 = np.ascontiguousarray(np.asarray(inputs["expert_w1"], dtype=np.float32))
    eb1 = np.ascontiguousarray(np.asarray(inputs["expert_b1"], dtype=np.float32))
    ew2 = np.ascontiguousarray(np.asarray(inputs["expert_w2"], dtype=np.float32))
    eb2 = np.ascontiguousarray(np.asarray(inputs["expert_b2"], dtype=np.float32))

    xf = x.reshape(N, D)
    xT = np.ascontiguousarray(xf.T)                       # [D, N]
    gwt = np.ascontiguousarray(
        gate_w.reshape(DCH, 128, E).transpose(1, 0, 2).reshape(128, DCH * E))

    perf = {}
    kw = dict(trace=trace)
    if trace and trace_cores is not None:
        kw["trace_cores"] = trace_cores

    # ---- pass 1: router ----
    nc1 = _get_pass1()
    in1 = [{"xt": np.ascontiguousarray(xT[:, c * TPC:(c + 1) * TPC]), "gwt": gwt}
           for c in range(NCORES)]
    r1 = run_bass_kernel_spmd(nc1, in1, core_ids=list(range(NCORES)), **kw)
    perf["pass1_ns"] = r1.exec_time_ns
    wfull = np.concatenate([r1.results[c]["wfull"] for c in range(NCORES)], axis=0)
    auxp = np.concatenate([r1.results[c]["auxp"] for c in range(NCORES)], axis=0)

    # ---- host dispatch (data movement only) ----
    idx = [np.nonzero(wfull[:, e])[0] for e in range(E)]
    maxc = max(len(i) for i in idx)
    C = max(TBLK, ((maxc + TBLK - 1) // TBLK) * TBLK)
    in2 = []
    for e in range(E):
        ie = idx[e]
        xg = np.zeros((D, C), np.float32)
        xg[:, :len(ie)] = xT[:, ie]
        wt = np.zeros(C, np.float32)
        wt[:len(ie)] = wfull[ie, e]
        in2.append({
            "xgt": xg,
            "w1": ew1[e],
            "w2": ew2[e],
            "b1t": np.ascontiguousarray(eb1[e].reshape(FCH, 128).T),
            "b2b": np.ascontiguousarray(np.broadcast_to(eb2[e], (128, D))),
            "wtok": np.ascontiguousarray(wt.reshape(C // 128, 128).T),
            "auxp": auxp,
        })

    # ---- pass 2: expert FFN ----
    has_b2 = bool(np.any(eb2))
    nc2 = _get_pass2(C, has_b2)
    r2 = run_bass_kernel_spmd(nc2, in2, core_ids=list(range(NCORES)), **kw)
    perf["pass2_ns"] = r2.exec_time_ns
    perf["C"] = C
    perf["r1"] = r1
    perf["r2"] = r2

    # ---- host combine (scatter-add of the two expert contributions) ----
    out = np.zeros((N, D), np.float32)
    for e in range(E):
        ie = idx[e]
        out[ie] += r2.results[e]["y"][:len(ie)]
    aux = np.float32(r2.results[0]["aux"][0, 0])
    return out.reshape(B, T, D), aux, perf


def kernel(**inputs):
    out, aux, _ = run(inputs, trace=bool(int(os.environ.get("KERNEL_TRACE", "0"))))
    return out, aux
